# revision 11
# baseline (speedup 1.0000x reference)
"""CollaborativeAttention (complex-valued, per-head mixed queries) on 8 trn2 cores.

Sharding: B*H = 24 (batch, head) units -> 3 heads per core.
  core c: batch b = c // 4, head block hb = c % 4 -> heads [3*hb, 3*hb+2].
Each core computes q/k projections for its batch (replicated within the
4-core batch group), v/cb projections for its head block only, then
scores+softmax+context for its 3 heads.

v5: every complex matmul (q/k/v/cb projections, scores) uses the 3-mult
Karatsuba form m1=ar@br, m2=ai@bi, m3=(ar+ai)@(br+bi); real=m1-m2,
imag=m3-m1-m2 -- 18 PE matmuls per tile-group instead of 24.  All matmul
operands are fp16 (same 1 cyc/row PE rate as fp32r, fp32 PSUM accumulate,
~8e-4 final relative error); the host ships h_sum=hr+hi and packed
[Wr|Wi|Wr+Wi] weight tiles in partition-major layout so every DMA line is
a contiguous run (no gather descriptors), each weight byte moves once,
and weights stream on the GpSimd DMA queue while hidden states stream
per-d-tile on the SP queue.

Phase P order: v/cb (sharing the projection PSUM banks), then q (the
block-0 per-head mixing is interleaved into its combines), then k with
ss-blocked groups -- so the first score matmuls, which need only the
ss=0 half of kT, issue as soon as the k ss=0 combines land while the PE
is still busy with k ss=1.

The softmax combine uses the product form
  Er = exp((m1-m2)/8 + cb) = exp(m1/8 + cb) * exp(-m2/8)
  Ei = exp(m3/8 + cb') * exp(-m1/8) * exp(-m2/8)
so ScalarE exps read PSUM directly (retiring each score accumulator right
after its matmul group: the three accumulators are single-buffered) and
the DVE does 3 cheap fp16 multiplies; the context accumulators
double-buffer, removing per-sj stalls.  The content bias (pre-scaled by
1/8, folded into the v/cb Karatsuba combine) rides the exp as a
per-partition ACT bias.  Softmax denominators come from a ones-column
appended to [vr | vi] in the context matmul's moving operand.  Mixing for
block n+1 is spread one n-tile per key tile through block n's score loop
(mq tiles are double-buffered so there is no WAR), so block boundaries
cost the PE nothing.

Layout notes: hidden is transposed on the HOST; scores are computed
transposed, sT[t, s], so probs land directly in the lhsT layout the
context matmul wants.  This walrus build encodes at most one sync-wait
per instruction, so a post-pass (_split_multi_waits) peels extra waits
onto NoOps.
"""

import sys

for _p in ("/opt/trn_rl_repo", "/root/.axon_site", "/root/.axon_site/_ro/trn_rl_repo",
           "/root/.axon_site/_ro/pypackages"):
    if _p not in sys.path:
        sys.path.append(_p)

import numpy as np

import concourse.bass as bass
import concourse.mybir as mybir
import concourse.tile as tile
from concourse.bass_utils import run_bass_kernel_spmd

B, S, D, H = 2, 1024, 768, 12
DK = DV = 768
DH = DV // H          # 64 per-head value dim
HPC = 3               # heads per core
N_CORES = 8
P = 128
ND = D // P           # 6 d-tiles (contraction)
NDK = DK // P         # 6 q/k n-tiles
NT = S // P           # 8 token tiles
SW = 512              # s-slice width for scores/projections
NS = S // SW          # 2 s-slices
VC = HPC * DH         # 192 value cols per core
VCB = VC + HPC        # 195: [Wv_j | Wcb_j/8] cols per Karatsuba part

FP = mybir.dt.float32
HF = mybir.dt.float16
AF = mybir.ActivationFunctionType
OP = mybir.AluOpType

TRACE = False
LAST_RESULTS = None

_compiled = None


def _split_multi_waits(nc):
    """The walrus build here encodes at most ONE sync-wait per instruction
    ("Too many sync wait commands" in setupSyncWait otherwise). Tile freely
    emits several. Split the extras onto single-wait NoOps that precede the
    instruction in the same engine stream."""
    for fn in nc.m.functions:
        for bb in fn.blocks:
            out = []
            for ins in bb.instructions:
                si = ins.sync_info
                if si is not None and len(si.on_wait) > 1:
                    waits = list(si.on_wait)
                    for j, w in enumerate(waits[:-1]):
                        nop = mybir.InstNoOp(name=f"{ins.name}-ws{j}",
                                             ins=[], outs=[])
                        nop.engine = ins.engine
                        nop.sync_info = mybir.SyncInfo(on_wait=[w], on_update=[])
                        out.append(nop)
                    ins.sync_info = mybir.SyncInfo(on_wait=[waits[-1]],
                                                   on_update=list(si.on_update))
                out.append(ins)
            bb.instructions = out


def _build(split_waits=True):
    """Build the SPMD Bass program (identical on all 8 cores)."""
    nc = bass.Bass(trn_type="TRN2")

    # all DRAM tensors are packed partition-major on the host: every DMA
    # line is a contiguous run, no gather descriptors
    hTr_d = nc.dram_tensor("hTr", [P, ND, S], HF, kind="ExternalInput")
    hTi_d = nc.dram_tensor("hTi", [P, ND, S], HF, kind="ExternalInput")
    hTs_d = nc.dram_tensor("hTs", [P, ND, S], HF, kind="ExternalInput")
    # packed [Wr | Wi | Wr+Wi] per (proj: 0=q 1=k, out n-tile, contraction d)
    wpk_d = nc.dram_tensor("wpk", [2, NDK, P, ND, 3 * P], HF,
                           kind="ExternalInput")
    # v/cb Karatsuba parts: [..., j, :] = [Wv_j | Wcb_j/8], j in (r, i, r+i)
    wvk_d = nc.dram_tensor("wvk", [P, ND, 3, VCB], HF, kind="ExternalInput")
    # [ones(P) | bv_r, 0(HPC) | bv_r+bv_i, 0(HPC)]: rank-1 bias rows for m1/m3
    bvk_d = nc.dram_tensor("bvk", [1, P + 2 * VCB], HF, kind="ExternalInput")
    mixv_d = nc.dram_tensor("mixv", [P, HPC * NDK * 3], FP, kind="ExternalInput")
    out_d = nc.dram_tensor("out", [2, HPC, NT, P, DH], FP, kind="ExternalOutput")

    with tile.TileContext(nc) as tc:
        with (
            tc.tile_pool(name="persist", bufs=1) as persist,
            tc.tile_pool(name="vstuff", bufs=1) as vstuff,
            tc.tile_pool(name="mqp", bufs=2) as mqp,
        ):
            # ---- persistent tensors -------------------------------------
            qTr = persist.tile([P, NDK, S], HF)
            qTi = persist.tile([P, NDK, S], HF)
            kTr = persist.tile([P, NDK, S], HF)
            kTi = persist.tile([P, NDK, S], HF)
            kTs = persist.tile([P, NDK, S], HF)

            bvk_sb = vstuff.tile([1, P + 2 * VCB], HF)
            nc.sync.dma_start(bvk_sb, bvk_d[:])
            # weights stream on the (otherwise idle) GpSimd DMA queue,
            # concurrent with the hidden-state stream on the SP queue
            wvk_sb = vstuff.tile([P, ND, 3, VCB], HF)
            nc.gpsimd.dma_start(wvk_sb, wvk_d[:])
            # per-head context rhs: [vr_h | vi_h | 1]
            vaug = [vstuff.tile([P, NT, 2 * DH + 1], HF, tag=f"vaug{h}",
                                name=f"vaug{h}")
                    for h in range(HPC)]
            for h in range(HPC):
                nc.vector.memset(vaug[h][:, :, 2 * DH], 1.0)
            # (cbr/8 | cbi/8) per head, flattened: col = tt*2*HPC + (0|HPC) + h
            cb8 = vstuff.tile([P, NT * 2 * HPC], FP)
            mixv = vstuff.tile([P, HPC * NDK * 3], FP)

            def emit_mix(h, ss, a, mq):
                """mixed query for head h, slice ss, n-tile a (fp16 DVE)."""
                mqr, mqi, mqs = mq
                ssl = slice(ss * SW, (ss + 1) * SW)
                mbase = (h * NDK + a) * 3
                mr = mixv[:, mbase:mbase + 1]
                mi = mixv[:, mbase + 1:mbase + 2]
                min_ = mixv[:, mbase + 2:mbase + 3]
                # mqr = qTr*mr - qTi*mi ; mqi = qTr*mi + qTi*mr ; mqs = mqr+mqi
                nc.vector.tensor_scalar_mul(mqr[:, a], qTr[:, a, ssl], mr)
                nc.vector.scalar_tensor_tensor(
                    mqr[:, a], qTi[:, a, ssl], min_, mqr[:, a],
                    op0=OP.mult, op1=OP.add)
                nc.vector.tensor_scalar_mul(mqi[:, a], qTr[:, a, ssl], mi)
                nc.vector.scalar_tensor_tensor(
                    mqi[:, a], qTi[:, a, ssl], mr, mqi[:, a],
                    op0=OP.mult, op1=OP.add)
                nc.vector.tensor_add(mqs[:, a], mqr[:, a], mqi[:, a])

            def alloc_mq():
                return (mqp.tile([P, NDK, SW], HF, tag="mqr", name="mqr"),
                        mqp.tile([P, NDK, SW], HF, tag="mqi", name="mqi"),
                        mqp.tile([P, NDK, SW], HF, tag="mqs", name="mqs"))

            # ---- phase P: projections -----------------------------------
            with (
                tc.tile_pool(name="hload", bufs=1) as hload,
                tc.tile_pool(name="wstream", bufs=2) as wstream,
                tc.tile_pool(name="pproj", bufs=1, space="PSUM") as pproj,
                tc.tile_pool(name="vstage", bufs=2) as vstage,
                tc.tile_pool(name="qkstage", bufs=2) as qkstage,
            ):
                # full-S resident hidden; per-d DMAs so compute starts early
                hr = hload.tile([P, ND, S], HF, tag="hr")
                hi = hload.tile([P, ND, S], HF, tag="hi")
                hs = hload.tile([P, ND, S], HF, tag="hs")
                wq0 = wstream.tile([P, ND, 3 * P], HF, tag="w")
                nc.gpsimd.dma_start(wq0, wpk_d[0, 0])
                for d in range(ND):
                    nc.sync.dma_start(hr[:, d], hTr_d[:, d])
                for d in range(ND):
                    nc.sync.dma_start(hi[:, d], hTi_d[:, d])
                for d in range(ND):
                    nc.sync.dma_start(hs[:, d], hTs_d[:, d])
                nc.sync.dma_start(mixv, mixv_d[:])

                # -- v / cb projections (Karatsuba, bias rows via bvk; PSUM
                #    banks shared with the q/k projection pool):
                #    m1 = hr@[Wv_r|Wcb_r/8] + 1@[bv_r|0]
                #    m2 = hi@[Wv_i|Wcb_i/8]
                #    m3 = hs@[Wv_s|Wcb_s/8] + 1@[bv_r+bv_i|0]
                for tt in range(NT):
                    tsl = slice(tt * P, (tt + 1) * P)
                    vm = [pproj.tile([P, SW], FP, tag=f"pp0{j}",
                                     name=f"vm{j}")[:, :VCB]
                          for j in range(3)]
                    nc.tensor.matmul(vm[0], bvk_sb[:, :P],
                                     bvk_sb[:, P:P + VCB],
                                     start=True, stop=False)
                    for d in range(ND):
                        nc.tensor.matmul(vm[0], hr[:, d, tsl], wvk_sb[:, d, 0],
                                         start=False, stop=(d == ND - 1))
                    for d in range(ND):
                        nc.tensor.matmul(vm[1], hi[:, d, tsl], wvk_sb[:, d, 1],
                                         start=(d == 0), stop=(d == ND - 1))
                    nc.tensor.matmul(vm[2], bvk_sb[:, :P], bvk_sb[:, P + VCB:],
                                     start=True, stop=False)
                    for d in range(ND):
                        nc.tensor.matmul(vm[2], hs[:, d, tsl], wvk_sb[:, d, 2],
                                         start=False, stop=(d == ND - 1))
                    # DVE can read only one PSUM operand; stage m2 in SBUF
                    sv = vstage.tile([P, VCB], FP, tag="sv")
                    t3 = vstage.tile([P, VCB], FP, tag="t3")
                    nc.scalar.activation(sv, vm[1], AF.Copy)
                    nc.vector.tensor_sub(t3, vm[2], sv)
                    for h in range(HPC):
                        c0 = h * DH
                        nc.vector.tensor_sub(vaug[h][:, tt, 0:DH],
                                             vm[0][:, c0:c0 + DH],
                                             sv[:, c0:c0 + DH])
                        nc.vector.tensor_sub(vaug[h][:, tt, DH:2 * DH],
                                             t3[:, c0:c0 + DH],
                                             vm[0][:, c0:c0 + DH])
                    cbc = tt * 2 * HPC
                    nc.vector.tensor_sub(cb8[:, cbc:cbc + HPC],
                                         vm[0][:, VC:VCB], sv[:, VC:VCB])
                    nc.vector.tensor_sub(cb8[:, cbc + HPC:cbc + 2 * HPC],
                                         t3[:, VC:VCB], vm[0][:, VC:VCB])

                # -- q/k projections, Karatsuba: per (proj, nt, ss)
                #    m1 = Wr.hr, m2 = Wi.hi, m3 = (Wr+Wi).(hr+hi)
                #    real = m1-m2, imag = m3-m1-m2, ksum = m3-2*m2
                mq0 = alloc_mq()  # block (h=0, ss=0) mixing, emitted in q loop

                def proj_combine(dst_r, dst_i, is_k, nt, ss, m1, m2, m3):
                    ssl = slice(ss * SW, (ss + 1) * SW)
                    s2 = qkstage.tile([P, SW], FP, tag="s2")
                    t3 = qkstage.tile([P, SW], FP, tag="t3")
                    nc.scalar.activation(s2, m2, AF.Copy)
                    nc.vector.tensor_sub(dst_r[:, nt, ssl], m1, s2)
                    nc.vector.tensor_sub(t3, m3, s2)
                    nc.vector.tensor_sub(dst_i[:, nt, ssl], t3, m1)
                    if is_k:
                        nc.vector.scalar_tensor_tensor(
                            kTs[:, nt, ssl], s2, -2.0, m3,
                            op0=OP.mult, op1=OP.add)
                    elif ss == 0:
                        emit_mix(0, 0, nt, mq0)

                def get_w(pi, nt):
                    if pi == 0 and nt == 0:
                        return wq0
                    w = wstream.tile([P, ND, 3 * P], HF, tag="w")
                    nc.gpsimd.dma_start(w, wpk_d[pi, nt])
                    return w

                # q: per-nt groups, both ss in flight (6 banks)
                for nt in range(NDK):
                    w = get_w(0, nt)
                    ps = {(ss, j): pproj.tile([P, SW], FP, tag=f"pp{ss}{j}",
                                              name=f"q{nt}_{ss}{j}")
                          for ss in range(NS) for j in range(3)}
                    for ss in range(NS):
                        ssl = slice(ss * SW, (ss + 1) * SW)
                        for j, src in enumerate((hr, hi, hs)):
                            for d in range(ND):
                                nc.tensor.matmul(
                                    ps[ss, j], w[:, d, j * P:(j + 1) * P],
                                    src[:, d, ssl],
                                    start=(d == 0), stop=(d == ND - 1))
                    for ss in range(NS):
                        proj_combine(qTr, qTi, False, nt, ss,
                                     ps[ss, 0], ps[ss, 1], ps[ss, 2])

                # k: ss-blocked nt-pair groups, so the ss=0 half of kT (all
                # the first four score key-tiles need) finishes first
                for ss in range(NS):
                    ssl = slice(ss * SW, (ss + 1) * SW)
                    for npr in range(NDK // 2):
                        nts = (2 * npr, 2 * npr + 1)
                        ws = {nt: get_w(1, nt) for nt in nts}
                        ps = {}
                        for ii, nt in enumerate(nts):
                            for j in range(3):
                                ps[nt, j] = pproj.tile(
                                    [P, SW], FP, tag=f"pp{ii}{j}",
                                    name=f"k{nt}_{ss}{j}")
                            for j, src in enumerate((hr, hi, hs)):
                                for d in range(ND):
                                    nc.tensor.matmul(
                                        ps[nt, j],
                                        ws[nt][:, d, j * P:(j + 1) * P],
                                        src[:, d, ssl],
                                        start=(d == 0), stop=(d == ND - 1))
                        for nt in nts:
                            proj_combine(kTr, kTi, True, nt, ss,
                                         ps[nt, 0], ps[nt, 1], ps[nt, 2])

            # ---- phase S: per-head scores -> softmax -> context ---------
            blocks = [(h, ss) for h in range(HPC) for ss in range(NS)]
            block_mq = {blocks[0]: mq0}
            with (
                tc.tile_pool(name="ep", bufs=1) as ep,
                tc.tile_pool(name="etile", bufs=2) as etile,
                tc.tile_pool(name="psc", bufs=1, space="PSUM") as psc,
                tc.tile_pool(name="pctx", bufs=2, space="PSUM") as pctx,
                tc.tile_pool(name="ctxs", bufs=4) as ctxs,
            ):
                for bi, (h, ss) in enumerate(blocks):
                    mqr, mqi, mqs = block_mq[h, ss]

                    Er = ep.tile([P, NT, SW], HF, tag="Er")
                    Ei = ep.tile([P, NT, SW], HF, tag="Ei")
                    for tt in range(NT):
                        tsl = slice(tt * P, (tt + 1) * P)
                        m1 = psc.tile([P, SW], FP, tag="m1", name="m1")
                        m2 = psc.tile([P, SW], FP, tag="m2", name="m2")
                        m3 = psc.tile([P, SW], FP, tag="m3", name="m3")
                        for d in range(NDK):
                            nc.tensor.matmul(m1, kTr[:, d, tsl], mqr[:, d],
                                             start=(d == 0), stop=(d == NDK - 1))
                        for d in range(NDK):
                            nc.tensor.matmul(m2, kTi[:, d, tsl], mqi[:, d],
                                             start=(d == 0), stop=(d == NDK - 1))
                        for d in range(NDK):
                            nc.tensor.matmul(m3, kTs[:, d, tsl], mqs[:, d],
                                             start=(d == 0), stop=(d == NDK - 1))
                        # product-form softmax numerators:
                        #   Er = exp(m1/8+cbr) * exp(-m2/8)
                        #   Ei = exp(m3/8+cbi) * exp(-m1/8) * exp(-m2/8)
                        E1 = etile.tile([P, SW], HF, tag="E1", name="E1")
                        E1m = etile.tile([P, SW], HF, tag="E1m", name="E1m")
                        E2 = etile.tile([P, SW], HF, tag="E2", name="E2")
                        E3 = etile.tile([P, SW], HF, tag="E3", name="E3")
                        cbc = tt * 2 * HPC
                        nc.scalar.activation(
                            E1, m1, AF.Exp,
                            bias=cb8[:, cbc + h:cbc + h + 1], scale=0.125)
                        nc.scalar.activation(E1m, m1, AF.Exp, scale=-0.125)
                        nc.scalar.activation(E2, m2, AF.Exp, scale=-0.125)
                        nc.scalar.activation(
                            E3, m3, AF.Exp,
                            bias=cb8[:, cbc + HPC + h:cbc + HPC + h + 1],
                            scale=0.125)
                        nc.vector.tensor_mul(Er[:, tt], E1, E2)
                        nc.vector.tensor_mul(E3, E3, E1m)
                        nc.vector.tensor_mul(Ei[:, tt], E3, E2)

                        # next block's mixing, spread one n-tile per key tile
                        # (mq is double-buffered so there is no WAR on it):
                        # by the time this block's scores finish, the next
                        # block's mixed queries are already in SBUF
                        if bi + 1 < len(blocks) and 1 <= tt <= NDK:
                            nh, nss = blocks[bi + 1]
                            if tt == 1:
                                block_mq[nh, nss] = alloc_mq()
                            emit_mix(nh, nss, tt - 1, block_mq[nh, nss])

                    # context: for each 128-row block of queries
                    for sj in range(SW // P):
                        st_idx = ss * (SW // P) + sj
                        qsl = slice(sj * P, (sj + 1) * P)
                        pcA = pctx.tile([P, 2 * DH + 1], FP, tag="pcA")
                        pcB = pctx.tile([P, 2 * DH + 1], FP, tag="pcB")
                        for tt in range(NT):
                            st, sp = tt == 0, tt == NT - 1
                            nc.tensor.matmul(pcA, Er[:, tt, qsl], vaug[h][:, tt],
                                             start=st, stop=sp)
                            nc.tensor.matmul(pcB, Ei[:, tt, qsl], vaug[h][:, tt],
                                             start=st, stop=sp)
                        rr = ctxs.tile([P, 1], FP, tag="rr")
                        ri = ctxs.tile([P, 1], FP, tag="ri")
                        nc.vector.reciprocal(rr, pcA[:, 2 * DH:2 * DH + 1])
                        nc.vector.reciprocal(ri, pcB[:, 2 * DH:2 * DH + 1])
                        # cr = A/sumr - Bvi/sumi ; ci = Avi/sumr + Bvr/sumi
                        tb = ctxs.tile([P, DH], FP, tag="tb")
                        td = ctxs.tile([P, DH], FP, tag="td")
                        cr = ctxs.tile([P, DH], FP, tag="cr")
                        ci = ctxs.tile([P, DH], FP, tag="ci")
                        nc.vector.tensor_scalar_mul(tb, pcB[:, DH:2 * DH], ri)
                        nc.vector.scalar_tensor_tensor(
                            cr, pcA[:, 0:DH], rr, tb, op0=OP.mult, op1=OP.subtract)
                        nc.vector.tensor_scalar_mul(td, pcB[:, 0:DH], ri)
                        nc.vector.scalar_tensor_tensor(
                            ci, pcA[:, DH:2 * DH], rr, td, op0=OP.mult, op1=OP.add)
                        nc.sync.dma_start(out_d[0, h, st_idx], cr)
                        nc.sync.dma_start(out_d[1, h, st_idx], ci)

    if split_waits:
        _split_multi_waits(nc)
    return nc


def _prep_shared(inputs):
    """Core-independent packed tensors (weights are replicated)."""
    f32 = lambda x: np.asarray(x, dtype=np.float32)
    c = np.ascontiguousarray

    # [proj, nt, P, d, 3P]: each partition row one contiguous (d, 3P) run
    wpk = np.empty((2, NDK, P, ND, 3 * P), np.float16)
    for pi, (wr_name, wi_name) in enumerate((("Wq_r", "Wq_i"), ("Wk_r", "Wk_i"))):
        wr, wi = f32(inputs[wr_name]), f32(inputs[wi_name])
        W3 = np.stack([wr, wi, wr + wi])              # [3, D, DK]
        wpk[pi] = (W3.reshape(3, ND, P, NDK, P)
                   .transpose(3, 2, 1, 0, 4).reshape(NDK, P, ND, 3 * P)
                   .astype(np.float16))

    def pack(x):  # [D, S] -> [P, ND, S] partition-major
        return c(x.reshape(ND, P, S).transpose(1, 0, 2).astype(np.float16))
    hT = {}
    for b in range(B):
        hr = f32(inputs["hidden_r"][b]).T             # [D, S]
        hi = f32(inputs["hidden_i"][b]).T
        hT[b] = (pack(hr), pack(hi), pack(hr + hi))
    return c(wpk), hT


def _prep_core_inputs(inputs, core, wpk, hT):
    hb = core % (N_CORES // B)
    heads = list(range(hb * HPC, (hb + 1) * HPC))
    cols = slice(hb * VC, (hb + 1) * VC)

    f32 = lambda x: np.asarray(x, dtype=np.float32)
    c = np.ascontiguousarray
    hr, hi, hs = hT[core // (N_CORES // B)]

    # v/cb Karatsuba parts with the content bias pre-scaled by 1/8
    wvr, wvi = f32(inputs["Wv_r"])[:, cols], f32(inputs["Wv_i"])[:, cols]
    cbr = f32(inputs["Wcb_r"])[:, heads] * 0.125
    cbi = f32(inputs["Wcb_i"])[:, heads] * 0.125
    wvk = np.stack([np.concatenate([wvr, cbr], axis=1),
                    np.concatenate([wvi, cbi], axis=1),
                    np.concatenate([wvr + wvi, cbr + cbi], axis=1)],
                   axis=1)                            # [D, 3, VCB]
    wvk = (wvk.reshape(ND, P, 3, VCB).transpose(1, 0, 2, 3)
           .astype(np.float16))                       # [P, ND, 3, VCB]
    bvr, bvi = f32(inputs["bv_r"])[cols], f32(inputs["bv_i"])[cols]
    z = np.zeros(HPC, np.float32)
    bvk = np.concatenate(
        [np.ones(P, np.float32), bvr, z, bvr + bvi, z]).astype(np.float16)

    mr = f32(inputs["mix_r"])[heads]     # [HPC, DK]
    mi = f32(inputs["mix_i"])[heads]
    # [P, (h, a, 3)]: partition-major, direct DMA
    mixv = np.stack([mr, mi, -mi], axis=-1).reshape(HPC, NDK, P, 3)
    mixv = mixv.transpose(2, 0, 1, 3).reshape(P, HPC * NDK * 3)

    return {
        "hTr": hr,
        "hTi": hi,
        "hTs": hs,
        "wpk": wpk,
        "wvk": c(wvk),
        "bvk": c(bvk.reshape(1, P + 2 * VCB)),
        "mixv": c(mixv),
    }


def kernel(**inputs):
    global _compiled, LAST_RESULTS
    if _compiled is None:
        _compiled = _build()
    nc = _compiled

    wpk, hT = _prep_shared(inputs)
    in_maps = [_prep_core_inputs(inputs, c, wpk, hT) for c in range(N_CORES)]
    res = run_bass_kernel_spmd(nc, in_maps, core_ids=list(range(N_CORES)),
                               trace=TRACE)
    LAST_RESULTS = res

    out = np.zeros((2, B, S, DV), np.float32)
    for core in range(N_CORES):
        b = core // (N_CORES // B)
        hb = core % (N_CORES // B)
        oc = res.results[core]["out"]  # [2, HPC, NT, P, DH]
        for j in range(HPC):
            h = hb * HPC + j
            out[:, b, :, h * DH:(h + 1) * DH] = oc[:, j].reshape(2, S, DH)
    return out


# revision 12
# speedup vs baseline: 1.0308x; 1.0308x over previous
"""CollaborativeAttention (complex-valued, per-head mixed queries) on 8 trn2 cores.

Sharding: B*H = 24 (batch, head) units -> 3 heads per core.
  core c: batch b = c // 4, head block hb = c % 4 -> heads [3*hb, 3*hb+2].
Each core computes q/k projections for its batch (replicated within the
4-core batch group), v/cb projections for its head block only, then
scores+softmax+context for its 3 heads.

v5: every complex matmul (q/k/v/cb projections, scores) uses the 3-mult
Karatsuba form m1=ar@br, m2=ai@bi, m3=(ar+ai)@(br+bi); real=m1-m2,
imag=m3-m1-m2 -- 18 PE matmuls per tile-group instead of 24.  All matmul
operands are fp16 (same 1 cyc/row PE rate as fp32r, fp32 PSUM accumulate,
~8e-4 final relative error); the host ships h_sum=hr+hi and packed
[Wr|Wi|Wr+Wi] weight tiles in partition-major layout so every DMA line is
a contiguous run (no gather descriptors), each weight byte moves once,
and weights stream on the GpSimd DMA queue while hidden states stream
per-d-tile on the SP queue.

Phase P order: v/cb (sharing the projection PSUM banks), then q (the
block-0 per-head mixing is interleaved into its combines), then k with
ss-blocked groups -- so the first score matmuls, which need only the
ss=0 half of kT, issue as soon as the k ss=0 combines land while the PE
is still busy with k ss=1.

The softmax combine uses the product form
  Er = exp((m1-m2)/8 + cb) = exp(m1/8 + cb) * exp(-m2/8)
  Ei = exp(m3/8 + cb') * exp(-m1/8) * exp(-m2/8)
so ScalarE exps read PSUM directly (retiring each score accumulator right
after its matmul group: the three accumulators are single-buffered) and
the DVE does 3 cheap fp16 multiplies; the context accumulators
double-buffer, removing per-sj stalls.  The content bias (pre-scaled by
1/8, folded into the v/cb Karatsuba combine) rides the exp as a
per-partition ACT bias.  Softmax denominators come from a ones-column
appended to [vr | vi] in the context matmul's moving operand.  Mixing for
block n+1 is spread one n-tile per key tile through block n's score loop
(mq tiles are double-buffered so there is no WAR), so block boundaries
cost the PE nothing.

Layout notes: hidden is transposed on the HOST; scores are computed
transposed, sT[t, s], so probs land directly in the lhsT layout the
context matmul wants.  This walrus build encodes at most one sync-wait
per instruction, so a post-pass (_split_multi_waits) peels extra waits
onto NoOps.
"""

import sys

for _p in ("/opt/trn_rl_repo", "/root/.axon_site", "/root/.axon_site/_ro/trn_rl_repo",
           "/root/.axon_site/_ro/pypackages"):
    if _p not in sys.path:
        sys.path.append(_p)

import numpy as np

import concourse.bass as bass
import concourse.mybir as mybir
import concourse.tile as tile
from concourse.bass_utils import run_bass_kernel_spmd

B, S, D, H = 2, 1024, 768, 12
DK = DV = 768
DH = DV // H          # 64 per-head value dim
HPC = 3               # heads per core
N_CORES = 8
P = 128
ND = D // P           # 6 d-tiles (contraction)
NDK = DK // P         # 6 q/k n-tiles
NT = S // P           # 8 token tiles
SW = 512              # s-slice width for scores/projections
NS = S // SW          # 2 s-slices
VC = HPC * DH         # 192 value cols per core
VCB = VC + HPC        # 195: [Wv_j | Wcb_j/8] cols per Karatsuba part

FP = mybir.dt.float32
HF = mybir.dt.float16
AF = mybir.ActivationFunctionType
OP = mybir.AluOpType

TRACE = False
LAST_RESULTS = None

_compiled = None


def _split_multi_waits(nc):
    """The walrus build here encodes at most ONE sync-wait per instruction
    ("Too many sync wait commands" in setupSyncWait otherwise). Tile freely
    emits several. Split the extras onto single-wait NoOps that precede the
    instruction in the same engine stream."""
    for fn in nc.m.functions:
        for bb in fn.blocks:
            out = []
            for ins in bb.instructions:
                si = ins.sync_info
                if si is not None and len(si.on_wait) > 1:
                    waits = list(si.on_wait)
                    for j, w in enumerate(waits[:-1]):
                        nop = mybir.InstNoOp(name=f"{ins.name}-ws{j}",
                                             ins=[], outs=[])
                        nop.engine = ins.engine
                        nop.sync_info = mybir.SyncInfo(on_wait=[w], on_update=[])
                        out.append(nop)
                    ins.sync_info = mybir.SyncInfo(on_wait=[waits[-1]],
                                                   on_update=list(si.on_update))
                out.append(ins)
            bb.instructions = out


def _build(split_waits=True):
    """Build the SPMD Bass program (identical on all 8 cores)."""
    nc = bass.Bass(trn_type="TRN2")

    # all DRAM tensors are packed partition-major on the host: every DMA
    # line is a contiguous run, no gather descriptors
    hTr_d = nc.dram_tensor("hTr", [P, ND, S], HF, kind="ExternalInput")
    hTi_d = nc.dram_tensor("hTi", [P, ND, S], HF, kind="ExternalInput")
    hTs_d = nc.dram_tensor("hTs", [P, ND, S], HF, kind="ExternalInput")
    # packed [Wr | Wi | Wr+Wi] per (proj: 0=q 1=k, out n-tile, contraction d)
    wpk_d = nc.dram_tensor("wpk", [2, NDK, P, ND, 3 * P], HF,
                           kind="ExternalInput")
    # v/cb Karatsuba parts: [..., j, :] = [Wv_j | Wcb_j/8], j in (r, i, r+i)
    wvk_d = nc.dram_tensor("wvk", [P, ND, 3, VCB], HF, kind="ExternalInput")
    # [ones(P) | bv_r, 0(HPC) | bv_r+bv_i, 0(HPC)]: rank-1 bias rows for m1/m3
    bvk_d = nc.dram_tensor("bvk", [1, P + 2 * VCB], HF, kind="ExternalInput")
    mixv_d = nc.dram_tensor("mixv", [P, HPC * NDK * 3], FP, kind="ExternalInput")
    out_d = nc.dram_tensor("out", [2, HPC, NT, P, DH], FP, kind="ExternalOutput")

    with tile.TileContext(nc) as tc:
        with (
            tc.tile_pool(name="persist", bufs=1) as persist,
            tc.tile_pool(name="vstuff", bufs=1) as vstuff,
            tc.tile_pool(name="mqp", bufs=2) as mqp,
        ):
            # ---- persistent tensors -------------------------------------
            qTr = persist.tile([P, NDK, S], HF)
            qTi = persist.tile([P, NDK, S], HF)
            kTr = persist.tile([P, NDK, S], HF)
            kTi = persist.tile([P, NDK, S], HF)
            kTs = persist.tile([P, NDK, S], HF)

            bvk_sb = vstuff.tile([1, P + 2 * VCB], HF)
            nc.sync.dma_start(bvk_sb, bvk_d[:])
            # weights stream on the (otherwise idle) GpSimd DMA queue,
            # concurrent with the hidden-state stream on the SP queue
            wvk_sb = vstuff.tile([P, ND, 3, VCB], HF)
            nc.gpsimd.dma_start(wvk_sb, wvk_d[:])
            # per-head context rhs: [vr_h | vi_h | 1]
            vaug = [vstuff.tile([P, NT, 2 * DH + 1], HF, tag=f"vaug{h}",
                                name=f"vaug{h}")
                    for h in range(HPC)]
            for h in range(HPC):
                nc.vector.memset(vaug[h][:, :, 2 * DH], 1.0)
            # (cbr/8 | cbi/8) per head, flattened: col = tt*2*HPC + (0|HPC) + h
            cb8 = vstuff.tile([P, NT * 2 * HPC], FP)
            mixv = vstuff.tile([P, HPC * NDK * 3], FP)

            def emit_mix(h, ss, a, mq):
                """mixed query for head h, slice ss, n-tile a (fp16 DVE)."""
                mqr, mqi, mqs = mq
                ssl = slice(ss * SW, (ss + 1) * SW)
                mbase = (h * NDK + a) * 3
                mr = mixv[:, mbase:mbase + 1]
                mi = mixv[:, mbase + 1:mbase + 2]
                min_ = mixv[:, mbase + 2:mbase + 3]
                # mqr = qTr*mr - qTi*mi ; mqi = qTr*mi + qTi*mr ; mqs = mqr+mqi
                nc.vector.tensor_scalar_mul(mqr[:, a], qTr[:, a, ssl], mr)
                nc.vector.scalar_tensor_tensor(
                    mqr[:, a], qTi[:, a, ssl], min_, mqr[:, a],
                    op0=OP.mult, op1=OP.add)
                nc.vector.tensor_scalar_mul(mqi[:, a], qTr[:, a, ssl], mi)
                nc.vector.scalar_tensor_tensor(
                    mqi[:, a], qTi[:, a, ssl], mr, mqi[:, a],
                    op0=OP.mult, op1=OP.add)
                nc.vector.tensor_add(mqs[:, a], mqr[:, a], mqi[:, a])

            def alloc_mq():
                return (mqp.tile([P, NDK, SW], HF, tag="mqr", name="mqr"),
                        mqp.tile([P, NDK, SW], HF, tag="mqi", name="mqi"),
                        mqp.tile([P, NDK, SW], HF, tag="mqs", name="mqs"))

            # ---- phase P: projections -----------------------------------
            with (
                tc.tile_pool(name="hload", bufs=1) as hload,
                tc.tile_pool(name="wstream", bufs=2) as wstream,
                tc.tile_pool(name="pproj", bufs=1, space="PSUM") as pproj,
                tc.tile_pool(name="vstage", bufs=2) as vstage,
                tc.tile_pool(name="qkstage", bufs=2) as qkstage,
            ):
                # full-S resident hidden; per-d DMAs so compute starts early
                hr = hload.tile([P, ND, S], HF, tag="hr")
                hi = hload.tile([P, ND, S], HF, tag="hi")
                hs = hload.tile([P, ND, S], HF, tag="hs")
                wq0 = wstream.tile([P, ND, 3 * P], HF, tag="w")
                nc.gpsimd.dma_start(wq0, wpk_d[0, 0])
                for d in range(ND):
                    nc.sync.dma_start(hr[:, d], hTr_d[:, d])
                for d in range(ND):
                    nc.sync.dma_start(hi[:, d], hTi_d[:, d])
                for d in range(ND):
                    nc.sync.dma_start(hs[:, d], hTs_d[:, d])
                nc.sync.dma_start(mixv, mixv_d[:])

                # -- v / cb projections (Karatsuba, bias rows via bvk; PSUM
                #    banks shared with the q/k projection pool):
                #    m1 = hr@[Wv_r|Wcb_r/8] + 1@[bv_r|0]
                #    m2 = hi@[Wv_i|Wcb_i/8]
                #    m3 = hs@[Wv_s|Wcb_s/8] + 1@[bv_r+bv_i|0]
                # q/k Karatsuba: per (proj, nt, ss)
                #    m1 = Wr.hr, m2 = Wi.hi, m3 = (Wr+Wi).(hr+hi)
                #    real = m1-m2, imag = m3-m1-m2, ksum = m3-2*m2
                mq0 = alloc_mq()  # block (h=0, ss=0) mixing, emitted in q loop

                def proj_combine(dst_r, dst_i, is_k, nt, ss, m1, m2, m3):
                    ssl = slice(ss * SW, (ss + 1) * SW)
                    s2 = qkstage.tile([P, SW], FP, tag="s2")
                    t3 = qkstage.tile([P, SW], FP, tag="t3")
                    nc.scalar.activation(s2, m2, AF.Copy)
                    nc.vector.tensor_sub(dst_r[:, nt, ssl], m1, s2)
                    nc.vector.tensor_sub(t3, m3, s2)
                    nc.vector.tensor_sub(dst_i[:, nt, ssl], t3, m1)
                    if is_k:
                        nc.vector.scalar_tensor_tensor(
                            kTs[:, nt, ssl], s2, -2.0, m3,
                            op0=OP.mult, op1=OP.add)
                    elif ss == 0:
                        emit_mix(0, 0, nt, mq0)

                def get_w(pi, nt):
                    if pi == 0 and nt == 0:
                        return wq0
                    w = wstream.tile([P, ND, 3 * P], HF, tag="w")
                    nc.gpsimd.dma_start(w, wpk_d[pi, nt])
                    return w

                def qk_mm_group(w, ps3, ss, srcs=(None,)):
                    ssl = slice(ss * SW, (ss + 1) * SW)
                    for j, src in enumerate((hr, hi, hs)):
                        for d in range(ND):
                            nc.tensor.matmul(
                                ps3[j], w[:, d, j * P:(j + 1) * P],
                                src[:, d, ssl],
                                start=(d == 0), stop=(d == ND - 1))

                # q nt0 ss1 runs on the pp1* banks (untouched by v/cb) and its
                # matmul groups are interleaved into the v/cb token loop to
                # fill the early hidden-DMA waits
                q0ps = {(1, j): pproj.tile([P, SW], FP, tag=f"pp1{j}",
                                           name=f"q0_1{j}")
                        for j in range(3)}

                def vcb_tt(tt):
                    tsl = slice(tt * P, (tt + 1) * P)
                    vm = [pproj.tile([P, SW], FP, tag=f"pp0{j}",
                                     name=f"vm{j}")[:, :VCB]
                          for j in range(3)]
                    nc.tensor.matmul(vm[0], bvk_sb[:, :P],
                                     bvk_sb[:, P:P + VCB],
                                     start=True, stop=False)
                    for d in range(ND):
                        nc.tensor.matmul(vm[0], hr[:, d, tsl], wvk_sb[:, d, 0],
                                         start=False, stop=(d == ND - 1))
                    for d in range(ND):
                        nc.tensor.matmul(vm[1], hi[:, d, tsl], wvk_sb[:, d, 1],
                                         start=(d == 0), stop=(d == ND - 1))
                    nc.tensor.matmul(vm[2], bvk_sb[:, :P], bvk_sb[:, P + VCB:],
                                     start=True, stop=False)
                    for d in range(ND):
                        nc.tensor.matmul(vm[2], hs[:, d, tsl], wvk_sb[:, d, 2],
                                         start=False, stop=(d == ND - 1))
                    # DVE can read only one PSUM operand; stage m2 in SBUF
                    sv = vstage.tile([P, VCB], FP, tag="sv")
                    t3 = vstage.tile([P, VCB], FP, tag="t3")
                    nc.scalar.activation(sv, vm[1], AF.Copy)
                    nc.vector.tensor_sub(t3, vm[2], sv)
                    for h in range(HPC):
                        c0 = h * DH
                        nc.vector.tensor_sub(vaug[h][:, tt, 0:DH],
                                             vm[0][:, c0:c0 + DH],
                                             sv[:, c0:c0 + DH])
                        nc.vector.tensor_sub(vaug[h][:, tt, DH:2 * DH],
                                             t3[:, c0:c0 + DH],
                                             vm[0][:, c0:c0 + DH])
                    cbc = tt * 2 * HPC
                    nc.vector.tensor_sub(cb8[:, cbc:cbc + HPC],
                                         vm[0][:, VC:VCB], sv[:, VC:VCB])
                    nc.vector.tensor_sub(cb8[:, cbc + HPC:cbc + 2 * HPC],
                                         t3[:, VC:VCB], vm[0][:, VC:VCB])

                ssl1 = slice(SW, 2 * SW)
                for tt in range(NT):
                    vcb_tt(tt)
                    if tt == 0:      # q nt0 ss1 m1: needs only wq0 + hr
                        for d in range(ND):
                            nc.tensor.matmul(q0ps[1, 0], wq0[:, d, 0:P],
                                             hr[:, d, ssl1],
                                             start=(d == 0), stop=(d == ND - 1))
                    elif tt == 2:    # m2: needs hi
                        for d in range(ND):
                            nc.tensor.matmul(q0ps[1, 1], wq0[:, d, P:2 * P],
                                             hi[:, d, ssl1],
                                             start=(d == 0), stop=(d == ND - 1))
                    elif tt == 4:    # m3: needs hs
                        for d in range(ND):
                            nc.tensor.matmul(q0ps[1, 2], wq0[:, d, 2 * P:3 * P],
                                             hs[:, d, ssl1],
                                             start=(d == 0), stop=(d == ND - 1))
                proj_combine(qTr, qTi, False, 0, 1,
                             q0ps[1, 0], q0ps[1, 1], q0ps[1, 2])

                # q then k: per-nt groups, ss-blocked within each group so the
                # first ss's combines hide under the second ss's matmuls
                for pi, (dst_r, dst_i) in enumerate(((qTr, qTi), (kTr, kTi))):
                    for nt in range(NDK):
                        w = get_w(pi, nt)
                        sss = (0,) if (pi, nt) == (0, 0) else (0, 1)
                        for ss in sss:
                            ps3 = [pproj.tile([P, SW], FP, tag=f"pp{ss}{j}",
                                              name=f"p{pi}{nt}_{ss}{j}")
                                   for j in range(3)]
                            qk_mm_group(w, ps3, ss)
                            proj_combine(dst_r, dst_i, pi == 1, nt, ss,
                                         ps3[0], ps3[1], ps3[2])

            # ---- phase S: per-head scores -> softmax -> context ---------
            blocks = [(h, ss) for h in range(HPC) for ss in range(NS)]
            block_mq = {blocks[0]: mq0}
            with (
                tc.tile_pool(name="ep", bufs=1) as ep,
                tc.tile_pool(name="etile", bufs=2) as etile,
                tc.tile_pool(name="psc", bufs=1, space="PSUM") as psc,
                tc.tile_pool(name="pctx", bufs=2, space="PSUM") as pctx,
                tc.tile_pool(name="ctxs", bufs=4) as ctxs,
            ):
                for bi, (h, ss) in enumerate(blocks):
                    mqr, mqi, mqs = block_mq[h, ss]

                    Er = ep.tile([P, NT, SW], HF, tag="Er")
                    Ei = ep.tile([P, NT, SW], HF, tag="Ei")
                    for tt in range(NT):
                        tsl = slice(tt * P, (tt + 1) * P)
                        m1 = psc.tile([P, SW], FP, tag="m1", name="m1")
                        m2 = psc.tile([P, SW], FP, tag="m2", name="m2")
                        m3 = psc.tile([P, SW], FP, tag="m3", name="m3")
                        for d in range(NDK):
                            nc.tensor.matmul(m1, kTr[:, d, tsl], mqr[:, d],
                                             start=(d == 0), stop=(d == NDK - 1))
                        for d in range(NDK):
                            nc.tensor.matmul(m2, kTi[:, d, tsl], mqi[:, d],
                                             start=(d == 0), stop=(d == NDK - 1))
                        for d in range(NDK):
                            nc.tensor.matmul(m3, kTs[:, d, tsl], mqs[:, d],
                                             start=(d == 0), stop=(d == NDK - 1))
                        # product-form softmax numerators:
                        #   Er = exp(m1/8+cbr) * exp(-m2/8)
                        #   Ei = exp(m3/8+cbi) * exp(-m1/8) * exp(-m2/8)
                        E1 = etile.tile([P, SW], HF, tag="E1", name="E1")
                        E1m = etile.tile([P, SW], HF, tag="E1m", name="E1m")
                        E2 = etile.tile([P, SW], HF, tag="E2", name="E2")
                        E3 = etile.tile([P, SW], HF, tag="E3", name="E3")
                        cbc = tt * 2 * HPC
                        nc.scalar.activation(
                            E1, m1, AF.Exp,
                            bias=cb8[:, cbc + h:cbc + h + 1], scale=0.125)
                        nc.scalar.activation(E1m, m1, AF.Exp, scale=-0.125)
                        nc.scalar.activation(E2, m2, AF.Exp, scale=-0.125)
                        nc.scalar.activation(
                            E3, m3, AF.Exp,
                            bias=cb8[:, cbc + HPC + h:cbc + HPC + h + 1],
                            scale=0.125)
                        nc.vector.tensor_mul(Er[:, tt], E1, E2)
                        nc.vector.tensor_mul(E3, E3, E1m)
                        nc.vector.tensor_mul(Ei[:, tt], E3, E2)

                        # next block's mixing, spread one n-tile per key tile
                        # (mq is double-buffered so there is no WAR on it):
                        # by the time this block's scores finish, the next
                        # block's mixed queries are already in SBUF
                        if bi + 1 < len(blocks) and 1 <= tt <= NDK:
                            nh, nss = blocks[bi + 1]
                            if tt == 1:
                                block_mq[nh, nss] = alloc_mq()
                            emit_mix(nh, nss, tt - 1, block_mq[nh, nss])

                    # context: for each 128-row block of queries
                    for sj in range(SW // P):
                        st_idx = ss * (SW // P) + sj
                        qsl = slice(sj * P, (sj + 1) * P)
                        pcA = pctx.tile([P, 2 * DH + 1], FP, tag="pcA")
                        pcB = pctx.tile([P, 2 * DH + 1], FP, tag="pcB")
                        for tt in range(NT):
                            st, sp = tt == 0, tt == NT - 1
                            nc.tensor.matmul(pcA, Er[:, tt, qsl], vaug[h][:, tt],
                                             start=st, stop=sp)
                            nc.tensor.matmul(pcB, Ei[:, tt, qsl], vaug[h][:, tt],
                                             start=st, stop=sp)
                        rr = ctxs.tile([P, 1], FP, tag="rr")
                        ri = ctxs.tile([P, 1], FP, tag="ri")
                        nc.vector.reciprocal(rr, pcA[:, 2 * DH:2 * DH + 1])
                        nc.vector.reciprocal(ri, pcB[:, 2 * DH:2 * DH + 1])
                        # cr = A/sumr - Bvi/sumi ; ci = Avi/sumr + Bvr/sumi
                        tb = ctxs.tile([P, DH], FP, tag="tb")
                        td = ctxs.tile([P, DH], FP, tag="td")
                        cr = ctxs.tile([P, DH], FP, tag="cr")
                        ci = ctxs.tile([P, DH], FP, tag="ci")
                        nc.vector.tensor_scalar_mul(tb, pcB[:, DH:2 * DH], ri)
                        nc.vector.scalar_tensor_tensor(
                            cr, pcA[:, 0:DH], rr, tb, op0=OP.mult, op1=OP.subtract)
                        nc.vector.tensor_scalar_mul(td, pcB[:, 0:DH], ri)
                        nc.vector.scalar_tensor_tensor(
                            ci, pcA[:, DH:2 * DH], rr, td, op0=OP.mult, op1=OP.add)
                        nc.sync.dma_start(out_d[0, h, st_idx], cr)
                        nc.sync.dma_start(out_d[1, h, st_idx], ci)

    if split_waits:
        _split_multi_waits(nc)
    return nc


def _prep_shared(inputs):
    """Core-independent packed tensors (weights are replicated)."""
    f32 = lambda x: np.asarray(x, dtype=np.float32)
    c = np.ascontiguousarray

    # [proj, nt, P, d, 3P]: each partition row one contiguous (d, 3P) run
    wpk = np.empty((2, NDK, P, ND, 3 * P), np.float16)
    for pi, (wr_name, wi_name) in enumerate((("Wq_r", "Wq_i"), ("Wk_r", "Wk_i"))):
        wr, wi = f32(inputs[wr_name]), f32(inputs[wi_name])
        W3 = np.stack([wr, wi, wr + wi])              # [3, D, DK]
        wpk[pi] = (W3.reshape(3, ND, P, NDK, P)
                   .transpose(3, 2, 1, 0, 4).reshape(NDK, P, ND, 3 * P)
                   .astype(np.float16))

    def pack(x):  # [D, S] -> [P, ND, S] partition-major
        return c(x.reshape(ND, P, S).transpose(1, 0, 2).astype(np.float16))
    hT = {}
    for b in range(B):
        hr = f32(inputs["hidden_r"][b]).T             # [D, S]
        hi = f32(inputs["hidden_i"][b]).T
        hT[b] = (pack(hr), pack(hi), pack(hr + hi))
    return c(wpk), hT


def _prep_core_inputs(inputs, core, wpk, hT):
    hb = core % (N_CORES // B)
    heads = list(range(hb * HPC, (hb + 1) * HPC))
    cols = slice(hb * VC, (hb + 1) * VC)

    f32 = lambda x: np.asarray(x, dtype=np.float32)
    c = np.ascontiguousarray
    hr, hi, hs = hT[core // (N_CORES // B)]

    # v/cb Karatsuba parts with the content bias pre-scaled by 1/8
    wvr, wvi = f32(inputs["Wv_r"])[:, cols], f32(inputs["Wv_i"])[:, cols]
    cbr = f32(inputs["Wcb_r"])[:, heads] * 0.125
    cbi = f32(inputs["Wcb_i"])[:, heads] * 0.125
    wvk = np.stack([np.concatenate([wvr, cbr], axis=1),
                    np.concatenate([wvi, cbi], axis=1),
                    np.concatenate([wvr + wvi, cbr + cbi], axis=1)],
                   axis=1)                            # [D, 3, VCB]
    wvk = (wvk.reshape(ND, P, 3, VCB).transpose(1, 0, 2, 3)
           .astype(np.float16))                       # [P, ND, 3, VCB]
    bvr, bvi = f32(inputs["bv_r"])[cols], f32(inputs["bv_i"])[cols]
    z = np.zeros(HPC, np.float32)
    bvk = np.concatenate(
        [np.ones(P, np.float32), bvr, z, bvr + bvi, z]).astype(np.float16)

    mr = f32(inputs["mix_r"])[heads]     # [HPC, DK]
    mi = f32(inputs["mix_i"])[heads]
    # [P, (h, a, 3)]: partition-major, direct DMA
    mixv = np.stack([mr, mi, -mi], axis=-1).reshape(HPC, NDK, P, 3)
    mixv = mixv.transpose(2, 0, 1, 3).reshape(P, HPC * NDK * 3)

    return {
        "hTr": hr,
        "hTi": hi,
        "hTs": hs,
        "wpk": wpk,
        "wvk": c(wvk),
        "bvk": c(bvk.reshape(1, P + 2 * VCB)),
        "mixv": c(mixv),
    }


def kernel(**inputs):
    global _compiled, LAST_RESULTS
    if _compiled is None:
        _compiled = _build()
    nc = _compiled

    wpk, hT = _prep_shared(inputs)
    in_maps = [_prep_core_inputs(inputs, c, wpk, hT) for c in range(N_CORES)]
    res = run_bass_kernel_spmd(nc, in_maps, core_ids=list(range(N_CORES)),
                               trace=TRACE)
    LAST_RESULTS = res

    out = np.zeros((2, B, S, DV), np.float32)
    for core in range(N_CORES):
        b = core // (N_CORES // B)
        hb = core % (N_CORES // B)
        oc = res.results[core]["out"]  # [2, HPC, NT, P, DH]
        for j in range(HPC):
            h = hb * HPC + j
            out[:, b, :, h * DH:(h + 1) * DH] = oc[:, j].reshape(2, S, DH)
    return out


# revision 13
# speedup vs baseline: 1.0511x; 1.0196x over previous
"""CollaborativeAttention (complex-valued, per-head mixed queries) on 8 trn2 cores.

Sharding: B*H = 24 (batch, head) units -> 3 heads per core.
  core c: batch b = c // 4, head block hb = c % 4 -> heads [3*hb, 3*hb+2].
Each core computes q/k projections for its batch (replicated within the
4-core batch group), v/cb projections for its head block only, then
scores+softmax+context for its 3 heads.

v5: every complex matmul (q/k/v/cb projections, scores) uses the 3-mult
Karatsuba form m1=ar@br, m2=ai@bi, m3=(ar+ai)@(br+bi); real=m1-m2,
imag=m3-m1-m2 -- 18 PE matmuls per tile-group instead of 24.  All matmul
operands are fp16 (same 1 cyc/row PE rate as fp32r, fp32 PSUM accumulate,
~8e-4 final relative error); the host ships h_sum=hr+hi and packed
[Wr|Wi|Wr+Wi] weight tiles in partition-major layout so every DMA line is
a contiguous run (no gather descriptors), each weight byte moves once,
and weights stream on the GpSimd DMA queue while hidden states stream
per-d-tile on the SP queue.

Phase P order: v/cb (sharing the projection PSUM banks), then q (the
block-0 per-head mixing is interleaved into its combines), then k with
ss-blocked groups -- so the first score matmuls, which need only the
ss=0 half of kT, issue as soon as the k ss=0 combines land while the PE
is still busy with k ss=1.

The softmax combine uses the product form
  Er = exp((m1-m2)/8 + cb) = exp(m1/8 + cb) * exp(-m2/8)
  Ei = exp(m3/8 + cb') * exp(-m1/8) * exp(-m2/8)
so ScalarE exps read PSUM directly (retiring each score accumulator right
after its matmul group: the three accumulators are single-buffered) and
the DVE does 3 cheap fp16 multiplies; the context accumulators
double-buffer, removing per-sj stalls.  The content bias (pre-scaled by
1/8, folded into the v/cb Karatsuba combine) rides the exp as a
per-partition ACT bias.  Softmax denominators come from a ones-column
appended to [vr | vi] in the context matmul's moving operand.  Mixing for
block n+1 is spread one n-tile per key tile through block n's score loop
(mq tiles are double-buffered so there is no WAR), so block boundaries
cost the PE nothing.

Layout notes: hidden is transposed on the HOST; scores are computed
transposed, sT[t, s], so probs land directly in the lhsT layout the
context matmul wants.  This walrus build encodes at most one sync-wait
per instruction, so a post-pass (_split_multi_waits) peels extra waits
onto NoOps.
"""

import sys

for _p in ("/opt/trn_rl_repo", "/root/.axon_site", "/root/.axon_site/_ro/trn_rl_repo",
           "/root/.axon_site/_ro/pypackages"):
    if _p not in sys.path:
        sys.path.append(_p)

import numpy as np

import concourse.bass as bass
import concourse.mybir as mybir
import concourse.tile as tile
from concourse.bass_utils import run_bass_kernel_spmd

B, S, D, H = 2, 1024, 768, 12
DK = DV = 768
DH = DV // H          # 64 per-head value dim
HPC = 3               # heads per core
N_CORES = 8
P = 128
ND = D // P           # 6 d-tiles (contraction)
NDK = DK // P         # 6 q/k n-tiles
NT = S // P           # 8 token tiles
SW = 512              # s-slice width for scores/projections
NS = S // SW          # 2 s-slices
VC = HPC * DH         # 192 value cols per core
VCB = VC + HPC        # 195: [Wv_j | Wcb_j/8] cols per Karatsuba part

FP = mybir.dt.float32
HF = mybir.dt.float16
AF = mybir.ActivationFunctionType
OP = mybir.AluOpType

TRACE = False
LAST_RESULTS = None

_compiled = None


def _split_multi_waits(nc):
    """The walrus build here encodes at most ONE sync-wait per instruction
    ("Too many sync wait commands" in setupSyncWait otherwise). Tile freely
    emits several. Split the extras onto single-wait NoOps that precede the
    instruction in the same engine stream."""
    for fn in nc.m.functions:
        for bb in fn.blocks:
            out = []
            for ins in bb.instructions:
                si = ins.sync_info
                if si is not None and len(si.on_wait) > 1:
                    waits = list(si.on_wait)
                    for j, w in enumerate(waits[:-1]):
                        nop = mybir.InstNoOp(name=f"{ins.name}-ws{j}",
                                             ins=[], outs=[])
                        nop.engine = ins.engine
                        nop.sync_info = mybir.SyncInfo(on_wait=[w], on_update=[])
                        out.append(nop)
                    ins.sync_info = mybir.SyncInfo(on_wait=[waits[-1]],
                                                   on_update=list(si.on_update))
                out.append(ins)
            bb.instructions = out


def _build(split_waits=True):
    """Build the SPMD Bass program (identical on all 8 cores)."""
    nc = bass.Bass(trn_type="TRN2")

    # all DRAM tensors are packed partition-major on the host: every DMA
    # line is a contiguous run, no gather descriptors
    hTr_d = nc.dram_tensor("hTr", [P, ND, S], HF, kind="ExternalInput")
    hTi_d = nc.dram_tensor("hTi", [P, ND, S], HF, kind="ExternalInput")
    hTs_d = nc.dram_tensor("hTs", [P, ND, S], HF, kind="ExternalInput")
    # packed [Wr | Wi | Wr+Wi] per (proj: 0=q 1=k, out n-tile, contraction d)
    wpk_d = nc.dram_tensor("wpk", [2, NDK, P, ND, 3 * P], HF,
                           kind="ExternalInput")
    # v/cb Karatsuba parts: [..., j, :] = [Wv_j | Wcb_j/8], j in (r, i, r+i)
    wvk_d = nc.dram_tensor("wvk", [P, ND, 3, VCB], HF, kind="ExternalInput")
    # [ones(P) | bv_r, 0(HPC) | bv_r+bv_i, 0(HPC)]: rank-1 bias rows for m1/m3
    bvk_d = nc.dram_tensor("bvk", [1, P + 2 * VCB], HF, kind="ExternalInput")
    mixv_d = nc.dram_tensor("mixv", [P, HPC * NDK * 3], FP, kind="ExternalInput")
    out_d = nc.dram_tensor("out", [2, HPC, NT, P, DH], FP, kind="ExternalOutput")

    with tile.TileContext(nc) as tc:
        with (
            tc.tile_pool(name="persist", bufs=1) as persist,
            tc.tile_pool(name="vstuff", bufs=1) as vstuff,
            tc.tile_pool(name="mqp", bufs=2) as mqp,
            tc.tile_pool(name="ep", bufs=1) as ep,
            tc.tile_pool(name="etile", bufs=2) as etile,
            tc.tile_pool(name="ctxs", bufs=4) as ctxs,
        ):
            # ---- persistent tensors -------------------------------------
            qTr = persist.tile([P, NDK, S], HF)
            qTi = persist.tile([P, NDK, S], HF)
            kTr = persist.tile([P, NDK, S], HF)
            kTi = persist.tile([P, NDK, S], HF)
            kTs = persist.tile([P, NDK, S], HF)

            bvk_sb = vstuff.tile([1, P + 2 * VCB], HF)
            nc.sync.dma_start(bvk_sb, bvk_d[:])
            # weights stream on the (otherwise idle) GpSimd DMA queue,
            # concurrent with the hidden-state stream on the SP queue
            wvk_sb = vstuff.tile([P, ND, 3, VCB], HF)
            nc.gpsimd.dma_start(wvk_sb, wvk_d[:])
            # per-head context rhs: [vr_h | vi_h | 1]
            vaug = [vstuff.tile([P, NT, 2 * DH + 1], HF, tag=f"vaug{h}",
                                name=f"vaug{h}")
                    for h in range(HPC)]
            for h in range(HPC):
                nc.vector.memset(vaug[h][:, :, 2 * DH], 1.0)
            # (cbr/8 | cbi/8) per head, flattened: col = tt*2*HPC + (0|HPC) + h
            cb8 = vstuff.tile([P, NT * 2 * HPC], FP)
            mixv = vstuff.tile([P, HPC * NDK * 3], FP)

            def emit_mix(h, ss, a, mq):
                """mixed query for head h, slice ss, n-tile a (fp16 DVE)."""
                mqr, mqi, mqs = mq
                ssl = slice(ss * SW, (ss + 1) * SW)
                mbase = (h * NDK + a) * 3
                mr = mixv[:, mbase:mbase + 1]
                mi = mixv[:, mbase + 1:mbase + 2]
                min_ = mixv[:, mbase + 2:mbase + 3]
                # mqr = qTr*mr - qTi*mi ; mqi = qTr*mi + qTi*mr ; mqs = mqr+mqi
                nc.vector.tensor_scalar_mul(mqr[:, a], qTr[:, a, ssl], mr)
                nc.vector.scalar_tensor_tensor(
                    mqr[:, a], qTi[:, a, ssl], min_, mqr[:, a],
                    op0=OP.mult, op1=OP.add)
                nc.vector.tensor_scalar_mul(mqi[:, a], qTr[:, a, ssl], mi)
                nc.vector.scalar_tensor_tensor(
                    mqi[:, a], qTi[:, a, ssl], mr, mqi[:, a],
                    op0=OP.mult, op1=OP.add)
                nc.vector.tensor_add(mqs[:, a], mqr[:, a], mqi[:, a])

            def alloc_mq():
                return (mqp.tile([P, NDK, SW], HF, tag="mqr", name="mqr"),
                        mqp.tile([P, NDK, SW], HF, tag="mqi", name="mqi"),
                        mqp.tile([P, NDK, SW], HF, tag="mqs", name="mqs"))

            # ---- phase P: projections -----------------------------------
            with (
                tc.tile_pool(name="hload", bufs=1) as hload,
                tc.tile_pool(name="wstream", bufs=2) as wstream,
                tc.tile_pool(name="pproj", bufs=1, space="PSUM") as pproj,
                tc.tile_pool(name="vstage", bufs=2) as vstage,
                tc.tile_pool(name="qkstage", bufs=2) as qkstage,
            ):
                # full-S resident hidden; per-d DMAs so compute starts early
                hr = hload.tile([P, ND, S], HF, tag="hr")
                hi = hload.tile([P, ND, S], HF, tag="hi")
                hs = hload.tile([P, ND, S], HF, tag="hs")
                wq0 = wstream.tile([P, ND, 3 * P], HF, tag="w")
                nc.gpsimd.dma_start(wq0, wpk_d[0, 0])
                nc.sync.dma_start(hr, hTr_d[:])
                nc.sync.dma_start(hi, hTi_d[:])
                nc.sync.dma_start(hs, hTs_d[:])
                nc.sync.dma_start(mixv, mixv_d[:])

                # -- v / cb projections (Karatsuba, bias rows via bvk; PSUM
                #    banks shared with the q/k projection pool):
                #    m1 = hr@[Wv_r|Wcb_r/8] + 1@[bv_r|0]
                #    m2 = hi@[Wv_i|Wcb_i/8]
                #    m3 = hs@[Wv_s|Wcb_s/8] + 1@[bv_r+bv_i|0]
                # q/k Karatsuba: per (proj, nt, ss)
                #    m1 = Wr.hr, m2 = Wi.hi, m3 = (Wr+Wi).(hr+hi)
                #    real = m1-m2, imag = m3-m1-m2, ksum = m3-2*m2
                mq0 = alloc_mq()  # block (h=0, ss=0) mixing, emitted in q loop

                def proj_combine(dst_r, dst_i, is_k, nt, ss, m1, m2, m3):
                    ssl = slice(ss * SW, (ss + 1) * SW)
                    s2 = qkstage.tile([P, SW], FP, tag="s2")
                    t3 = qkstage.tile([P, SW], FP, tag="t3")
                    nc.scalar.activation(s2, m2, AF.Copy)
                    nc.vector.tensor_sub(dst_r[:, nt, ssl], m1, s2)
                    nc.vector.tensor_sub(t3, m3, s2)
                    nc.vector.tensor_sub(dst_i[:, nt, ssl], t3, m1)
                    if is_k:
                        nc.vector.scalar_tensor_tensor(
                            kTs[:, nt, ssl], s2, -2.0, m3,
                            op0=OP.mult, op1=OP.add)
                    elif ss == 0:
                        emit_mix(0, 0, nt, mq0)

                def get_w(pi, nt):
                    if pi == 0 and nt == 0:
                        return wq0
                    w = wstream.tile([P, ND, 3 * P], HF, tag="w")
                    nc.gpsimd.dma_start(w, wpk_d[pi, nt])
                    return w

                def qk_mm_group(w, ps3, ss, srcs=(None,)):
                    ssl = slice(ss * SW, (ss + 1) * SW)
                    for j, src in enumerate((hr, hi, hs)):
                        for d in range(ND):
                            nc.tensor.matmul(
                                ps3[j], w[:, d, j * P:(j + 1) * P],
                                src[:, d, ssl],
                                start=(d == 0), stop=(d == ND - 1))

                # v/cb token tiles run in PAIRS across all six banks, with
                # the m1 groups of both tiles first: m1 needs only hr, so the
                # PE has work while the hi/hs DMAs are still landing.  All
                # three accumulators are staged to SBUF (sm1 via ScalarE) so
                # the banks retire as soon as the stage copies finish.
                def vcb_mm(j, tt, vm):
                    tsl = slice(tt * P, (tt + 1) * P)
                    src_ = (hr, hi, hs)[j]
                    if j != 1:
                        nc.tensor.matmul(vm[j], bvk_sb[:, :P],
                                         bvk_sb[:, P + (j // 2) * VCB:
                                                P + (j // 2 + 1) * VCB],
                                         start=True, stop=False)
                    for d in range(ND):
                        nc.tensor.matmul(vm[j], src_[:, d, tsl], wvk_sb[:, d, j],
                                         start=(j == 1 and d == 0),
                                         stop=(d == ND - 1))

                def vcb_combine(tt, vm):
                    sm1 = vstage.tile([P, VCB], FP, tag="sm1")
                    sv = vstage.tile([P, VCB], FP, tag="sv")
                    t3 = vstage.tile([P, VCB], FP, tag="t3")
                    nc.scalar.activation(sm1, vm[0], AF.Copy)
                    nc.scalar.activation(sv, vm[1], AF.Copy)
                    nc.vector.tensor_sub(t3, vm[2], sv)
                    for h in range(HPC):
                        c0 = h * DH
                        nc.vector.tensor_sub(vaug[h][:, tt, 0:DH],
                                             sm1[:, c0:c0 + DH],
                                             sv[:, c0:c0 + DH])
                        nc.vector.tensor_sub(vaug[h][:, tt, DH:2 * DH],
                                             t3[:, c0:c0 + DH],
                                             sm1[:, c0:c0 + DH])
                    cbc = tt * 2 * HPC
                    nc.vector.tensor_sub(cb8[:, cbc:cbc + HPC],
                                         sm1[:, VC:VCB], sv[:, VC:VCB])
                    nc.vector.tensor_sub(cb8[:, cbc + HPC:cbc + 2 * HPC],
                                         t3[:, VC:VCB], sm1[:, VC:VCB])

                for tp in range(NT // 2):
                    tts = (2 * tp, 2 * tp + 1)
                    vms = {tt: [pproj.tile([P, SW], FP, tag=f"pp{ii}{j}",
                                           name=f"vm{ii}{j}")[:, :VCB]
                                for j in range(3)]
                           for ii, tt in enumerate(tts)}
                    for j in range(3):
                        for tt in tts:
                            vcb_mm(j, tt, vms[tt])
                    for tt in tts:
                        vcb_combine(tt, vms[tt])

                # q then k: per-nt groups, ss-blocked within each group so the
                # first ss's combines hide under the second ss's matmuls
                for pi, (dst_r, dst_i) in enumerate(((qTr, qTi), (kTr, kTi))):
                    for nt in range(NDK):
                        w = get_w(pi, nt)
                        sss = (0, 1)
                        for ss in sss:
                            ps3 = [pproj.tile([P, SW], FP, tag=f"pp{ss}{j}",
                                              name=f"p{pi}{nt}_{ss}{j}")
                                   for j in range(3)]
                            qk_mm_group(w, ps3, ss)
                            proj_combine(dst_r, dst_i, pi == 1, nt, ss,
                                         ps3[0], ps3[1], ps3[2])

            # ---- phase S: per-head scores -> softmax -> context ---------
            blocks = [(h, ss) for h in range(HPC) for ss in range(NS)]
            block_mq = {blocks[0]: mq0}
            with (
                tc.tile_pool(name="psc", bufs=1, space="PSUM") as psc,
                tc.tile_pool(name="pctx", bufs=2, space="PSUM") as pctx,
            ):
                for bi, (h, ss) in enumerate(blocks):
                    mqr, mqi, mqs = block_mq[h, ss]

                    Er = ep.tile([P, NT, SW], HF, tag="Er")
                    Ei = ep.tile([P, NT, SW], HF, tag="Ei")
                    for tt in range(NT):
                        tsl = slice(tt * P, (tt + 1) * P)
                        m1 = psc.tile([P, SW], FP, tag="m1", name="m1")
                        m2 = psc.tile([P, SW], FP, tag="m2", name="m2")
                        m3 = psc.tile([P, SW], FP, tag="m3", name="m3")
                        for d in range(NDK):
                            nc.tensor.matmul(m1, kTr[:, d, tsl], mqr[:, d],
                                             start=(d == 0), stop=(d == NDK - 1))
                        for d in range(NDK):
                            nc.tensor.matmul(m2, kTi[:, d, tsl], mqi[:, d],
                                             start=(d == 0), stop=(d == NDK - 1))
                        for d in range(NDK):
                            nc.tensor.matmul(m3, kTs[:, d, tsl], mqs[:, d],
                                             start=(d == 0), stop=(d == NDK - 1))
                        # product-form softmax numerators:
                        #   Er = exp(m1/8+cbr) * exp(-m2/8)
                        #   Ei = exp(m3/8+cbi) * exp(-m1/8) * exp(-m2/8)
                        E1 = etile.tile([P, SW], HF, tag="E1", name="E1")
                        E1m = etile.tile([P, SW], HF, tag="E1m", name="E1m")
                        E2 = etile.tile([P, SW], HF, tag="E2", name="E2")
                        E3 = etile.tile([P, SW], HF, tag="E3", name="E3")
                        cbc = tt * 2 * HPC
                        nc.scalar.activation(
                            E1, m1, AF.Exp,
                            bias=cb8[:, cbc + h:cbc + h + 1], scale=0.125)
                        nc.scalar.activation(E1m, m1, AF.Exp, scale=-0.125)
                        nc.scalar.activation(E2, m2, AF.Exp, scale=-0.125)
                        nc.scalar.activation(
                            E3, m3, AF.Exp,
                            bias=cb8[:, cbc + HPC + h:cbc + HPC + h + 1],
                            scale=0.125)
                        nc.vector.tensor_mul(Er[:, tt], E1, E2)
                        nc.vector.tensor_mul(E3, E3, E1m)
                        nc.vector.tensor_mul(Ei[:, tt], E3, E2)

                        # next block's mixing, spread one n-tile per key tile
                        # (mq is double-buffered so there is no WAR on it):
                        # by the time this block's scores finish, the next
                        # block's mixed queries are already in SBUF
                        if bi + 1 < len(blocks) and 1 <= tt <= NDK:
                            nh, nss = blocks[bi + 1]
                            if tt == 1:
                                block_mq[nh, nss] = alloc_mq()
                            emit_mix(nh, nss, tt - 1, block_mq[nh, nss])

                    # context: for each 128-row block of queries
                    for sj in range(SW // P):
                        st_idx = ss * (SW // P) + sj
                        qsl = slice(sj * P, (sj + 1) * P)
                        pcA = pctx.tile([P, 2 * DH + 1], FP, tag="pcA")
                        pcB = pctx.tile([P, 2 * DH + 1], FP, tag="pcB")
                        for tt in range(NT):
                            st, sp = tt == 0, tt == NT - 1
                            nc.tensor.matmul(pcA, Er[:, tt, qsl], vaug[h][:, tt],
                                             start=st, stop=sp)
                            nc.tensor.matmul(pcB, Ei[:, tt, qsl], vaug[h][:, tt],
                                             start=st, stop=sp)
                        rr = ctxs.tile([P, 1], FP, tag="rr")
                        ri = ctxs.tile([P, 1], FP, tag="ri")
                        nc.vector.reciprocal(rr, pcA[:, 2 * DH:2 * DH + 1])
                        nc.vector.reciprocal(ri, pcB[:, 2 * DH:2 * DH + 1])
                        # cr = A/sumr - Bvi/sumi ; ci = Avi/sumr + Bvr/sumi
                        tb = ctxs.tile([P, DH], FP, tag="tb")
                        td = ctxs.tile([P, DH], FP, tag="td")
                        cr = ctxs.tile([P, DH], FP, tag="cr")
                        ci = ctxs.tile([P, DH], FP, tag="ci")
                        nc.vector.tensor_scalar_mul(tb, pcB[:, DH:2 * DH], ri)
                        nc.vector.scalar_tensor_tensor(
                            cr, pcA[:, 0:DH], rr, tb, op0=OP.mult, op1=OP.subtract)
                        nc.vector.tensor_scalar_mul(td, pcB[:, 0:DH], ri)
                        nc.vector.scalar_tensor_tensor(
                            ci, pcA[:, DH:2 * DH], rr, td, op0=OP.mult, op1=OP.add)
                        nc.sync.dma_start(out_d[0, h, st_idx], cr)
                        nc.sync.dma_start(out_d[1, h, st_idx], ci)

    if split_waits:
        _split_multi_waits(nc)
    return nc


def _prep_shared(inputs):
    """Core-independent packed tensors (weights are replicated)."""
    f32 = lambda x: np.asarray(x, dtype=np.float32)
    c = np.ascontiguousarray

    # [proj, nt, P, d, 3P]: each partition row one contiguous (d, 3P) run
    wpk = np.empty((2, NDK, P, ND, 3 * P), np.float16)
    for pi, (wr_name, wi_name) in enumerate((("Wq_r", "Wq_i"), ("Wk_r", "Wk_i"))):
        wr, wi = f32(inputs[wr_name]), f32(inputs[wi_name])
        W3 = np.stack([wr, wi, wr + wi])              # [3, D, DK]
        wpk[pi] = (W3.reshape(3, ND, P, NDK, P)
                   .transpose(3, 2, 1, 0, 4).reshape(NDK, P, ND, 3 * P)
                   .astype(np.float16))

    def pack(x):  # [D, S] -> [P, ND, S] partition-major
        return c(x.reshape(ND, P, S).transpose(1, 0, 2).astype(np.float16))
    hT = {}
    for b in range(B):
        hr = f32(inputs["hidden_r"][b]).T             # [D, S]
        hi = f32(inputs["hidden_i"][b]).T
        hT[b] = (pack(hr), pack(hi), pack(hr + hi))
    return c(wpk), hT


def _prep_core_inputs(inputs, core, wpk, hT):
    hb = core % (N_CORES // B)
    heads = list(range(hb * HPC, (hb + 1) * HPC))
    cols = slice(hb * VC, (hb + 1) * VC)

    f32 = lambda x: np.asarray(x, dtype=np.float32)
    c = np.ascontiguousarray
    hr, hi, hs = hT[core // (N_CORES // B)]

    # v/cb Karatsuba parts with the content bias pre-scaled by 1/8
    wvr, wvi = f32(inputs["Wv_r"])[:, cols], f32(inputs["Wv_i"])[:, cols]
    cbr = f32(inputs["Wcb_r"])[:, heads] * 0.125
    cbi = f32(inputs["Wcb_i"])[:, heads] * 0.125
    wvk = np.stack([np.concatenate([wvr, cbr], axis=1),
                    np.concatenate([wvi, cbi], axis=1),
                    np.concatenate([wvr + wvi, cbr + cbi], axis=1)],
                   axis=1)                            # [D, 3, VCB]
    wvk = (wvk.reshape(ND, P, 3, VCB).transpose(1, 0, 2, 3)
           .astype(np.float16))                       # [P, ND, 3, VCB]
    bvr, bvi = f32(inputs["bv_r"])[cols], f32(inputs["bv_i"])[cols]
    z = np.zeros(HPC, np.float32)
    bvk = np.concatenate(
        [np.ones(P, np.float32), bvr, z, bvr + bvi, z]).astype(np.float16)

    mr = f32(inputs["mix_r"])[heads]     # [HPC, DK]
    mi = f32(inputs["mix_i"])[heads]
    # [P, (h, a, 3)]: partition-major, direct DMA
    mixv = np.stack([mr, mi, -mi], axis=-1).reshape(HPC, NDK, P, 3)
    mixv = mixv.transpose(2, 0, 1, 3).reshape(P, HPC * NDK * 3)

    return {
        "hTr": hr,
        "hTi": hi,
        "hTs": hs,
        "wpk": wpk,
        "wvk": c(wvk),
        "bvk": c(bvk.reshape(1, P + 2 * VCB)),
        "mixv": c(mixv),
    }


def kernel(**inputs):
    global _compiled, LAST_RESULTS
    if _compiled is None:
        _compiled = _build()
    nc = _compiled

    wpk, hT = _prep_shared(inputs)
    in_maps = [_prep_core_inputs(inputs, c, wpk, hT) for c in range(N_CORES)]
    res = run_bass_kernel_spmd(nc, in_maps, core_ids=list(range(N_CORES)),
                               trace=TRACE)
    LAST_RESULTS = res

    out = np.zeros((2, B, S, DV), np.float32)
    for core in range(N_CORES):
        b = core // (N_CORES // B)
        hb = core % (N_CORES // B)
        oc = res.results[core]["out"]  # [2, HPC, NT, P, DH]
        for j in range(HPC):
            h = hb * HPC + j
            out[:, b, :, h * DH:(h + 1) * DH] = oc[:, j].reshape(2, S, DH)
    return out


# revision 14
# speedup vs baseline: 1.0718x; 1.0197x over previous
"""CollaborativeAttention (complex-valued, per-head mixed queries) on 8 trn2 cores.

Sharding: B*H = 24 (batch, head) units -> 3 heads per core.
  core c: batch b = c // 4, head block hb = c % 4 -> heads [3*hb, 3*hb+2].
Each core computes q/k projections for its batch (replicated within the
4-core batch group), v/cb projections for its head block only, then
scores+softmax+context for its 3 heads.

v5: every complex matmul (q/k/v/cb projections, scores) uses the 3-mult
Karatsuba form m1=ar@br, m2=ai@bi, m3=(ar+ai)@(br+bi); real=m1-m2,
imag=m3-m1-m2 -- 18 PE matmuls per tile-group instead of 24.  All matmul
operands are fp16 (same 1 cyc/row PE rate as fp32r, fp32 PSUM accumulate,
~8e-4 final relative error); the host ships h_sum=hr+hi and packed
[Wr|Wi|Wr+Wi] weight tiles in partition-major layout so every DMA line is
a contiguous run (no gather descriptors), each weight byte moves once,
and weights stream on the GpSimd DMA queue while hidden states stream
per-d-tile on the SP queue.

Phase P order: v/cb (sharing the projection PSUM banks), then q (the
block-0 per-head mixing is interleaved into its combines), then k with
ss-blocked groups -- so the first score matmuls, which need only the
ss=0 half of kT, issue as soon as the k ss=0 combines land while the PE
is still busy with k ss=1.

The softmax combine uses the product form
  Er = exp((m1-m2)/8 + cb) = exp(m1/8 + cb) * exp(-m2/8)
  Ei = exp(m3/8 + cb') * exp(-m1/8) * exp(-m2/8)
so ScalarE exps read PSUM directly (retiring each score accumulator right
after its matmul group: the three accumulators are single-buffered) and
the DVE does 3 cheap fp16 multiplies; the context accumulators
double-buffer, removing per-sj stalls.  The content bias (pre-scaled by
1/8, folded into the v/cb Karatsuba combine) rides the exp as a
per-partition ACT bias.  Softmax denominators come from a ones-column
appended to [vr | vi] in the context matmul's moving operand.  Mixing for
block n+1 is spread one n-tile per key tile through block n's score loop
(mq tiles are double-buffered so there is no WAR), so block boundaries
cost the PE nothing.

Layout notes: hidden is transposed on the HOST; scores are computed
transposed, sT[t, s], so probs land directly in the lhsT layout the
context matmul wants.  This walrus build encodes at most one sync-wait
per instruction, so a post-pass (_split_multi_waits) peels extra waits
onto NoOps.
"""

import sys

for _p in ("/opt/trn_rl_repo", "/root/.axon_site", "/root/.axon_site/_ro/trn_rl_repo",
           "/root/.axon_site/_ro/pypackages"):
    if _p not in sys.path:
        sys.path.append(_p)

import numpy as np

import concourse.bass as bass
import concourse.mybir as mybir
import concourse.tile as tile
from concourse.bass_utils import run_bass_kernel_spmd

B, S, D, H = 2, 1024, 768, 12
DK = DV = 768
DH = DV // H          # 64 per-head value dim
HPC = 3               # heads per core
N_CORES = 8
P = 128
ND = D // P           # 6 d-tiles (contraction)
NDK = DK // P         # 6 q/k n-tiles
NT = S // P           # 8 token tiles
SW = 512              # s-slice width for scores/projections
NS = S // SW          # 2 s-slices
VC = HPC * DH         # 192 value cols per core
VCB = VC + HPC        # 195: [Wv_j | Wcb_j/8] cols per Karatsuba part

FP = mybir.dt.float32
HF = mybir.dt.float16
AF = mybir.ActivationFunctionType
OP = mybir.AluOpType

TRACE = False
LAST_RESULTS = None

_compiled = None


def _split_multi_waits(nc):
    """The walrus build here encodes at most ONE sync-wait per instruction
    ("Too many sync wait commands" in setupSyncWait otherwise). Tile freely
    emits several. Split the extras onto single-wait NoOps that precede the
    instruction in the same engine stream."""
    for fn in nc.m.functions:
        for bb in fn.blocks:
            out = []
            for ins in bb.instructions:
                si = ins.sync_info
                if si is not None and len(si.on_wait) > 1:
                    waits = list(si.on_wait)
                    for j, w in enumerate(waits[:-1]):
                        nop = mybir.InstNoOp(name=f"{ins.name}-ws{j}",
                                             ins=[], outs=[])
                        nop.engine = ins.engine
                        nop.sync_info = mybir.SyncInfo(on_wait=[w], on_update=[])
                        out.append(nop)
                    ins.sync_info = mybir.SyncInfo(on_wait=[waits[-1]],
                                                   on_update=list(si.on_update))
                out.append(ins)
            bb.instructions = out


def _build(split_waits=True):
    """Build the SPMD Bass program (identical on all 8 cores)."""
    nc = bass.Bass(trn_type="TRN2")

    # all DRAM tensors are packed partition-major on the host: every DMA
    # line is a contiguous run, no gather descriptors
    hTr_d = nc.dram_tensor("hTr", [P, ND, S], HF, kind="ExternalInput")
    hTi_d = nc.dram_tensor("hTi", [P, ND, S], HF, kind="ExternalInput")
    hTs_d = nc.dram_tensor("hTs", [P, ND, S], HF, kind="ExternalInput")
    # packed [Wr | Wi | Wr+Wi] per (proj: 0=q 1=k, out n-tile, contraction d)
    wpk_d = nc.dram_tensor("wpk", [2, NDK, P, ND, 3 * P], HF,
                           kind="ExternalInput")
    # v/cb Karatsuba parts: [..., j, :] = [Wv_j | Wcb_j/8], j in (r, i, r+i)
    wvk_d = nc.dram_tensor("wvk", [P, ND, 3, VCB], HF, kind="ExternalInput")
    # [ones(P) | bv_r, 0(HPC) | bv_r+bv_i, 0(HPC)]: rank-1 bias rows for m1/m3
    bvk_d = nc.dram_tensor("bvk", [1, P + 2 * VCB], HF, kind="ExternalInput")
    mixv_d = nc.dram_tensor("mixv", [P, HPC * NDK * 3], FP, kind="ExternalInput")
    out_d = nc.dram_tensor("out", [2, HPC, NT, P, DH], FP, kind="ExternalOutput")

    with tile.TileContext(nc) as tc:
        with (
            tc.tile_pool(name="persist", bufs=1) as persist,
            tc.tile_pool(name="vstuff", bufs=1) as vstuff,
            tc.tile_pool(name="mqp", bufs=2) as mqp,
            tc.tile_pool(name="ep", bufs=1) as ep,
            tc.tile_pool(name="etile", bufs=2) as etile,
            tc.tile_pool(name="ctxs", bufs=4) as ctxs,
        ):
            # ---- persistent tensors -------------------------------------
            qTr = persist.tile([P, NDK, S], HF)
            qTi = persist.tile([P, NDK, S], HF)
            kTr = persist.tile([P, NDK, S], HF)
            kTi = persist.tile([P, NDK, S], HF)
            kTs = persist.tile([P, NDK, S], HF)

            bvk_sb = vstuff.tile([1, P + 2 * VCB], HF)
            nc.sync.dma_start(bvk_sb, bvk_d[:])
            # weights stream on the (otherwise idle) GpSimd DMA queue,
            # concurrent with the hidden-state stream on the SP queue
            wvk_sb = vstuff.tile([P, ND, 3, VCB], HF)
            nc.gpsimd.dma_start(wvk_sb, wvk_d[:])
            # per-head context rhs: [vr_h | vi_h | 1]
            vaug = [vstuff.tile([P, NT, 2 * DH + 1], HF, tag=f"vaug{h}",
                                name=f"vaug{h}")
                    for h in range(HPC)]
            for h in range(HPC):
                nc.vector.memset(vaug[h][:, :, 2 * DH], 1.0)
            # (cbr/8 | cbi/8) per head, flattened: col = tt*2*HPC + (0|HPC) + h
            cb8 = vstuff.tile([P, NT * 2 * HPC], FP)
            mixv = vstuff.tile([P, HPC * NDK * 3], FP)

            def emit_mix(h, ss, a, mq):
                """mixed query for head h, slice ss, n-tile a (fp16 DVE)."""
                mqr, mqi, mqs = mq
                ssl = slice(ss * SW, (ss + 1) * SW)
                mbase = (h * NDK + a) * 3
                mr = mixv[:, mbase:mbase + 1]
                mi = mixv[:, mbase + 1:mbase + 2]
                min_ = mixv[:, mbase + 2:mbase + 3]
                # mqr = qTr*mr - qTi*mi ; mqi = qTr*mi + qTi*mr ; mqs = mqr+mqi
                nc.vector.tensor_scalar_mul(mqr[:, a], qTr[:, a, ssl], mr)
                nc.vector.scalar_tensor_tensor(
                    mqr[:, a], qTi[:, a, ssl], min_, mqr[:, a],
                    op0=OP.mult, op1=OP.add)
                nc.vector.tensor_scalar_mul(mqi[:, a], qTr[:, a, ssl], mi)
                nc.vector.scalar_tensor_tensor(
                    mqi[:, a], qTi[:, a, ssl], mr, mqi[:, a],
                    op0=OP.mult, op1=OP.add)
                nc.vector.tensor_add(mqs[:, a], mqr[:, a], mqi[:, a])

            def alloc_mq():
                return (mqp.tile([P, NDK, SW], HF, tag="mqr", name="mqr"),
                        mqp.tile([P, NDK, SW], HF, tag="mqi", name="mqi"),
                        mqp.tile([P, NDK, SW], HF, tag="mqs", name="mqs"))

            # ---- phase P: projections -----------------------------------
            with (
                tc.tile_pool(name="hload", bufs=1) as hload,
                tc.tile_pool(name="wstream", bufs=2) as wstream,
                tc.tile_pool(name="pproj", bufs=1, space="PSUM") as pproj,
                tc.tile_pool(name="vstage", bufs=2) as vstage,
                tc.tile_pool(name="qkstage", bufs=2) as qkstage,
            ):
                # full-S resident hidden; per-d DMAs so compute starts early
                hr = hload.tile([P, ND, S], HF, tag="hr")
                hi = hload.tile([P, ND, S], HF, tag="hi")
                hs = hload.tile([P, ND, S], HF, tag="hs")
                wq0 = wstream.tile([P, ND, 3 * P], HF, tag="w")
                nc.gpsimd.dma_start(wq0, wpk_d[0, 0])
                # S-sliced, slice-major: v/cb token pair p needs only cols
                # 256p:256p+256, so the PE starts ~10 us earlier than with
                # whole-tensor loads
                for lo, hi_ in ((0, 256), (256, 512), (512, S)):
                    for t, td in ((hr, hTr_d), (hi, hTi_d), (hs, hTs_d)):
                        nc.sync.dma_start(t[:, :, lo:hi_], td[:, :, lo:hi_])
                nc.sync.dma_start(mixv, mixv_d[:])

                # -- v / cb projections (Karatsuba, bias rows via bvk; PSUM
                #    banks shared with the q/k projection pool):
                #    m1 = hr@[Wv_r|Wcb_r/8] + 1@[bv_r|0]
                #    m2 = hi@[Wv_i|Wcb_i/8]
                #    m3 = hs@[Wv_s|Wcb_s/8] + 1@[bv_r+bv_i|0]
                # q/k Karatsuba: per (proj, nt, ss)
                #    m1 = Wr.hr, m2 = Wi.hi, m3 = (Wr+Wi).(hr+hi)
                #    real = m1-m2, imag = m3-m1-m2, ksum = m3-2*m2
                mq0 = alloc_mq()  # block (h=0, ss=0) mixing, emitted in q loop

                def proj_combine(dst_r, dst_i, is_k, nt, ss, m1, m2, m3,
                                 staged=False):
                    ssl = slice(ss * SW, (ss + 1) * SW)
                    s2 = qkstage.tile([P, SW], FP, tag="s2")
                    t3 = qkstage.tile([P, SW], FP, tag="t3")
                    nc.scalar.activation(s2, m2, AF.Copy)
                    if staged:
                        # stage all accumulators so the PSUM banks retire
                        # immediately: the first phase-S score matmul reuses
                        # them and must not wait for these combines
                        s1 = qkstage.tile([P, SW], FP, tag="s1")
                        s3 = qkstage.tile([P, SW], FP, tag="s3")
                        nc.scalar.activation(s1, m1, AF.Copy)
                        nc.scalar.activation(s3, m3, AF.Copy)
                        m1, m3 = s1, s3
                    nc.vector.tensor_sub(dst_r[:, nt, ssl], m1, s2)
                    nc.vector.tensor_sub(t3, m3, s2)
                    nc.vector.tensor_sub(dst_i[:, nt, ssl], t3, m1)
                    if is_k:
                        nc.vector.scalar_tensor_tensor(
                            kTs[:, nt, ssl], s2, -2.0, m3,
                            op0=OP.mult, op1=OP.add)
                    elif ss == 0:
                        emit_mix(0, 0, nt, mq0)

                def get_w(pi, nt):
                    if pi == 0 and nt == 0:
                        return wq0
                    w = wstream.tile([P, ND, 3 * P], HF, tag="w")
                    nc.gpsimd.dma_start(w, wpk_d[pi, nt])
                    return w

                def qk_mm_group(w, ps3, ss, srcs=(None,)):
                    ssl = slice(ss * SW, (ss + 1) * SW)
                    for j, src in enumerate((hr, hi, hs)):
                        for d in range(ND):
                            nc.tensor.matmul(
                                ps3[j], w[:, d, j * P:(j + 1) * P],
                                src[:, d, ssl],
                                start=(d == 0), stop=(d == ND - 1))

                # v/cb token tiles run in PAIRS across all six banks, with
                # the m1 groups of both tiles first: m1 needs only hr, so the
                # PE has work while the hi/hs DMAs are still landing.  All
                # three accumulators are staged to SBUF (sm1 via ScalarE) so
                # the banks retire as soon as the stage copies finish.
                def vcb_mm(j, tt, vm):
                    tsl = slice(tt * P, (tt + 1) * P)
                    src_ = (hr, hi, hs)[j]
                    if j != 1:
                        nc.tensor.matmul(vm[j], bvk_sb[:, :P],
                                         bvk_sb[:, P + (j // 2) * VCB:
                                                P + (j // 2 + 1) * VCB],
                                         start=True, stop=False)
                    for d in range(ND):
                        nc.tensor.matmul(vm[j], src_[:, d, tsl], wvk_sb[:, d, j],
                                         start=(j == 1 and d == 0),
                                         stop=(d == ND - 1))

                def vcb_combine(tt, vm):
                    sm1 = vstage.tile([P, VCB], FP, tag="sm1")
                    sv = vstage.tile([P, VCB], FP, tag="sv")
                    t3 = vstage.tile([P, VCB], FP, tag="t3")
                    nc.scalar.activation(sm1, vm[0], AF.Copy)
                    nc.scalar.activation(sv, vm[1], AF.Copy)
                    nc.vector.tensor_sub(t3, vm[2], sv)
                    for h in range(HPC):
                        c0 = h * DH
                        nc.vector.tensor_sub(vaug[h][:, tt, 0:DH],
                                             sm1[:, c0:c0 + DH],
                                             sv[:, c0:c0 + DH])
                        nc.vector.tensor_sub(vaug[h][:, tt, DH:2 * DH],
                                             t3[:, c0:c0 + DH],
                                             sm1[:, c0:c0 + DH])
                    cbc = tt * 2 * HPC
                    nc.vector.tensor_sub(cb8[:, cbc:cbc + HPC],
                                         sm1[:, VC:VCB], sv[:, VC:VCB])
                    nc.vector.tensor_sub(cb8[:, cbc + HPC:cbc + 2 * HPC],
                                         t3[:, VC:VCB], sm1[:, VC:VCB])

                for tp in range(NT // 2):
                    tts = (2 * tp, 2 * tp + 1)
                    vms = {tt: [pproj.tile([P, SW], FP, tag=f"pp{ii}{j}",
                                           name=f"vm{ii}{j}")[:, :VCB]
                                for j in range(3)]
                           for ii, tt in enumerate(tts)}
                    for j in range(3):
                        for tt in tts:
                            vcb_mm(j, tt, vms[tt])
                    for tt in tts:
                        vcb_combine(tt, vms[tt])

                # q then k: per-nt groups, ss-blocked within each group so the
                # first ss's combines hide under the second ss's matmuls
                for pi, (dst_r, dst_i) in enumerate(((qTr, qTi), (kTr, kTi))):
                    for nt in range(NDK):
                        w = get_w(pi, nt)
                        sss = (0, 1)
                        for ss in sss:
                            ps3 = [pproj.tile([P, SW], FP, tag=f"pp{ss}{j}",
                                              name=f"p{pi}{nt}_{ss}{j}")
                                   for j in range(3)]
                            qk_mm_group(w, ps3, ss)
                            proj_combine(dst_r, dst_i, pi == 1, nt, ss,
                                         ps3[0], ps3[1], ps3[2],
                                         staged=(pi, nt) == (1, NDK - 1))

            # ---- phase S: per-head scores -> softmax -> context ---------
            blocks = [(h, ss) for h in range(HPC) for ss in range(NS)]
            block_mq = {blocks[0]: mq0}
            with (
                tc.tile_pool(name="psc", bufs=1, space="PSUM") as psc,
                tc.tile_pool(name="pctx", bufs=2, space="PSUM") as pctx,
            ):
                for bi, (h, ss) in enumerate(blocks):
                    mqr, mqi, mqs = block_mq[h, ss]

                    Er = ep.tile([P, NT, SW], HF, tag="Er")
                    Ei = ep.tile([P, NT, SW], HF, tag="Ei")
                    for tt in range(NT):
                        tsl = slice(tt * P, (tt + 1) * P)
                        m1 = psc.tile([P, SW], FP, tag="m1", name="m1")
                        m2 = psc.tile([P, SW], FP, tag="m2", name="m2")
                        m3 = psc.tile([P, SW], FP, tag="m3", name="m3")
                        for d in range(NDK):
                            nc.tensor.matmul(m1, kTr[:, d, tsl], mqr[:, d],
                                             start=(d == 0), stop=(d == NDK - 1))
                        for d in range(NDK):
                            nc.tensor.matmul(m2, kTi[:, d, tsl], mqi[:, d],
                                             start=(d == 0), stop=(d == NDK - 1))
                        for d in range(NDK):
                            nc.tensor.matmul(m3, kTs[:, d, tsl], mqs[:, d],
                                             start=(d == 0), stop=(d == NDK - 1))
                        # product-form softmax numerators:
                        #   Er = exp(m1/8+cbr) * exp(-m2/8)
                        #   Ei = exp(m3/8+cbi) * exp(-m1/8) * exp(-m2/8)
                        E1 = etile.tile([P, SW], HF, tag="E1", name="E1")
                        E1m = etile.tile([P, SW], HF, tag="E1m", name="E1m")
                        E2 = etile.tile([P, SW], HF, tag="E2", name="E2")
                        E3 = etile.tile([P, SW], HF, tag="E3", name="E3")
                        cbc = tt * 2 * HPC
                        nc.scalar.activation(
                            E1, m1, AF.Exp,
                            bias=cb8[:, cbc + h:cbc + h + 1], scale=0.125)
                        nc.scalar.activation(E1m, m1, AF.Exp, scale=-0.125)
                        nc.scalar.activation(E2, m2, AF.Exp, scale=-0.125)
                        nc.scalar.activation(
                            E3, m3, AF.Exp,
                            bias=cb8[:, cbc + HPC + h:cbc + HPC + h + 1],
                            scale=0.125)
                        nc.vector.tensor_mul(Er[:, tt], E1, E2)
                        nc.vector.tensor_mul(E3, E3, E1m)
                        nc.vector.tensor_mul(Ei[:, tt], E3, E2)

                        # next block's mixing, spread one n-tile per key tile
                        # (mq is double-buffered so there is no WAR on it):
                        # by the time this block's scores finish, the next
                        # block's mixed queries are already in SBUF
                        if bi + 1 < len(blocks) and 1 <= tt <= NDK:
                            nh, nss = blocks[bi + 1]
                            if tt == 1:
                                block_mq[nh, nss] = alloc_mq()
                            emit_mix(nh, nss, tt - 1, block_mq[nh, nss])

                    # context: for each 128-row block of queries
                    for sj in range(SW // P):
                        st_idx = ss * (SW // P) + sj
                        qsl = slice(sj * P, (sj + 1) * P)
                        pcA = pctx.tile([P, 2 * DH + 1], FP, tag="pcA")
                        pcB = pctx.tile([P, 2 * DH + 1], FP, tag="pcB")
                        for tt in range(NT):
                            st, sp = tt == 0, tt == NT - 1
                            nc.tensor.matmul(pcA, Er[:, tt, qsl], vaug[h][:, tt],
                                             start=st, stop=sp)
                            nc.tensor.matmul(pcB, Ei[:, tt, qsl], vaug[h][:, tt],
                                             start=st, stop=sp)
                        rr = ctxs.tile([P, 1], FP, tag="rr")
                        ri = ctxs.tile([P, 1], FP, tag="ri")
                        nc.vector.reciprocal(rr, pcA[:, 2 * DH:2 * DH + 1])
                        nc.vector.reciprocal(ri, pcB[:, 2 * DH:2 * DH + 1])
                        # cr = A/sumr - Bvi/sumi ; ci = Avi/sumr + Bvr/sumi
                        tb = ctxs.tile([P, DH], FP, tag="tb")
                        td = ctxs.tile([P, DH], FP, tag="td")
                        cr = ctxs.tile([P, DH], FP, tag="cr")
                        ci = ctxs.tile([P, DH], FP, tag="ci")
                        nc.vector.tensor_scalar_mul(tb, pcB[:, DH:2 * DH], ri)
                        nc.vector.scalar_tensor_tensor(
                            cr, pcA[:, 0:DH], rr, tb, op0=OP.mult, op1=OP.subtract)
                        nc.vector.tensor_scalar_mul(td, pcB[:, 0:DH], ri)
                        nc.vector.scalar_tensor_tensor(
                            ci, pcA[:, DH:2 * DH], rr, td, op0=OP.mult, op1=OP.add)
                        nc.sync.dma_start(out_d[0, h, st_idx], cr)
                        nc.sync.dma_start(out_d[1, h, st_idx], ci)

    if split_waits:
        _split_multi_waits(nc)
    return nc


def _prep_shared(inputs):
    """Core-independent packed tensors (weights are replicated)."""
    f32 = lambda x: np.asarray(x, dtype=np.float32)
    c = np.ascontiguousarray

    # [proj, nt, P, d, 3P]: each partition row one contiguous (d, 3P) run
    wpk = np.empty((2, NDK, P, ND, 3 * P), np.float16)
    for pi, (wr_name, wi_name) in enumerate((("Wq_r", "Wq_i"), ("Wk_r", "Wk_i"))):
        wr, wi = f32(inputs[wr_name]), f32(inputs[wi_name])
        W3 = np.stack([wr, wi, wr + wi])              # [3, D, DK]
        wpk[pi] = (W3.reshape(3, ND, P, NDK, P)
                   .transpose(3, 2, 1, 0, 4).reshape(NDK, P, ND, 3 * P)
                   .astype(np.float16))

    def pack(x):  # [D, S] -> [P, ND, S] partition-major
        return c(x.reshape(ND, P, S).transpose(1, 0, 2).astype(np.float16))
    hT = {}
    for b in range(B):
        hr = f32(inputs["hidden_r"][b]).T             # [D, S]
        hi = f32(inputs["hidden_i"][b]).T
        hT[b] = (pack(hr), pack(hi), pack(hr + hi))
    return c(wpk), hT


def _prep_core_inputs(inputs, core, wpk, hT):
    hb = core % (N_CORES // B)
    heads = list(range(hb * HPC, (hb + 1) * HPC))
    cols = slice(hb * VC, (hb + 1) * VC)

    f32 = lambda x: np.asarray(x, dtype=np.float32)
    c = np.ascontiguousarray
    hr, hi, hs = hT[core // (N_CORES // B)]

    # v/cb Karatsuba parts with the content bias pre-scaled by 1/8
    wvr, wvi = f32(inputs["Wv_r"])[:, cols], f32(inputs["Wv_i"])[:, cols]
    cbr = f32(inputs["Wcb_r"])[:, heads] * 0.125
    cbi = f32(inputs["Wcb_i"])[:, heads] * 0.125
    wvk = np.stack([np.concatenate([wvr, cbr], axis=1),
                    np.concatenate([wvi, cbi], axis=1),
                    np.concatenate([wvr + wvi, cbr + cbi], axis=1)],
                   axis=1)                            # [D, 3, VCB]
    wvk = (wvk.reshape(ND, P, 3, VCB).transpose(1, 0, 2, 3)
           .astype(np.float16))                       # [P, ND, 3, VCB]
    bvr, bvi = f32(inputs["bv_r"])[cols], f32(inputs["bv_i"])[cols]
    z = np.zeros(HPC, np.float32)
    bvk = np.concatenate(
        [np.ones(P, np.float32), bvr, z, bvr + bvi, z]).astype(np.float16)

    mr = f32(inputs["mix_r"])[heads]     # [HPC, DK]
    mi = f32(inputs["mix_i"])[heads]
    # [P, (h, a, 3)]: partition-major, direct DMA
    mixv = np.stack([mr, mi, -mi], axis=-1).reshape(HPC, NDK, P, 3)
    mixv = mixv.transpose(2, 0, 1, 3).reshape(P, HPC * NDK * 3)

    return {
        "hTr": hr,
        "hTi": hi,
        "hTs": hs,
        "wpk": wpk,
        "wvk": c(wvk),
        "bvk": c(bvk.reshape(1, P + 2 * VCB)),
        "mixv": c(mixv),
    }


def kernel(**inputs):
    global _compiled, LAST_RESULTS
    if _compiled is None:
        _compiled = _build()
    nc = _compiled

    wpk, hT = _prep_shared(inputs)
    in_maps = [_prep_core_inputs(inputs, c, wpk, hT) for c in range(N_CORES)]
    res = run_bass_kernel_spmd(nc, in_maps, core_ids=list(range(N_CORES)),
                               trace=TRACE)
    LAST_RESULTS = res

    out = np.zeros((2, B, S, DV), np.float32)
    for core in range(N_CORES):
        b = core // (N_CORES // B)
        hb = core % (N_CORES // B)
        oc = res.results[core]["out"]  # [2, HPC, NT, P, DH]
        for j in range(HPC):
            h = hb * HPC + j
            out[:, b, :, h * DH:(h + 1) * DH] = oc[:, j].reshape(2, S, DH)
    return out
